# revision 38
# baseline (speedup 1.0000x reference)
"""Trainium2 Bass kernel for nn_Attention (dense transformer attention block).

Full-input contract: kernel(**inputs) takes the unsharded inputs and returns
the full output. 8 NeuronCores: tensor-parallel over head groups (4 heads) x
data-parallel over batch (2); core c = b*4 + g. Per core: q/k/v projections
for its head group, RoPE, causal flash-style attention (transposed-P layout,
softmax without max-subtraction), partial o_proj with its rows of Wo; the 4
partials per batch element are summed on the host (the all-reduce of the
row-sharded o_proj).

Optimizations over the first working version (sim 351us -> 287us; HW
366us -> 317us at 2.4GHz):
 - softmax denominator via DVE exp-sum accumulation + ONE ones-matmul
   partition reduce on PE per (head, q-chunk) (the GpSimd
   partition_all_reduce used before costs 3.6us per call on HW and
   serialized the normalize chains: replacing it won -47us). Each head's
   den/recip/ctx-mul chain is woven after the next head's first two QK
   units so the in-order PE absorbs the esum-chain wait.
 - A1 alternates psum bank quads per sc; A2 is d-major (evict chains
   overlap the next d's matmuls); sc0 xt DMAs split per 128KB piece.
 - NOTE: fp8e4m3 PV (DoubleRow) was tried and REJECTED: fp8 noise on
   v/pexp does not average down (signal shrinks with the same 1/sqrt(n)
   as the noise); measured ~3.5% output error vs the 2% budget.
 - NOTE: HW runs sometimes execute with PE at 2.0GHz (P0 power state)
   instead of 2.4GHz -- wall times inflate ~20% run-to-run; check the
   median matmul duration (379ns warm @2.4 vs 454ns @2.0) to compare.
 - causal diagonal blocks trimmed to their valid column range (QK, exp,
   mask add, PV all operate on [128j:512) only); diag blocks ordered last
   so each head's tail exp/esum chain is short.
 - deep PV pend pipeline (8 blocks): exp-gated PV matmuls never stall the
   in-order PE mid-stream; they drain as pure-PE bursts.
 - A2 projection split into two d-pair passes on psum banks {0,1} only
   (same cycles), freeing banks 2/3 for a 4-deep sps rotation and a second
   ctx bank in the attention streams.
 - ctx normalization multiply reads the PSUM bank directly (no scalar
   copy); reciprocal via the fast approx DVE op.
 - o_proj software-pipelined over grouped PSUM banks with stationary
   reuse; qc=2's o_proj deferred into the qc=3 stream as PE filler (qc=3
   has no projection filler and is exp-latency-bound); final group's
   evict+DMA split in halves across Act/DVE copies and both DMA issue
   queues to shorten the end drain.
 - first xt/wv DMAs split into 128KB pieces, xt prefetch pipelined, sc=0
   xt tiles reused by A2(0); fp16 output (host accumulates in fp32).
 - dummy warmup matmuls burn the PE clock-ramp during the DMA-idle start.

The Q/K projection work (phase A2) is interleaved into the attention stream
(phase B) as PE filler: B-qc only needs Q/K columns of chunks <= qc, so
A2-sc(qc+1) runs alongside B-qc, hiding the exp/softmax latency chains.

Matmul dtype configurable (DTYPE): fp16 default (~7e-4 rel err), f32r
fallback (~3.5e-4).
"""
import contextlib
import numpy as np
import concourse.bass as bass
from concourse import bacc, bass_isa
import concourse.mybir as mybir
import concourse.tile as tile
from concourse.bass_utils import run_bass_kernel_spmd

F32 = mybir.dt.float32
F32R = mybir.dt.float32r
F16 = mybir.dt.float16
BF16 = mybir.dt.bfloat16
F8 = mybir.dt.float8e4
DR = mybir.MatmulPerfMode.DoubleRow
EXP = mybir.ActivationFunctionType.Exp
MMDT = {"f32r": F32R, "f16": F16, "bf16": BF16}
# fp8 PV (DoubleRow) was tried and REJECTED: per-element fp8e4m3 noise on
# v/pexp does not average down (signal shrinks with the same 1/sqrt(n) as
# the noise) -- measured ~3.5%% output error vs the 2%% budget. Keep f16.
PV8 = False
DR_ON = False

S = 2048
HID = 2048
D = 128
GH = 4            # heads per core
GW = GH * D       # 512
NCORES = 8
SC = S // 512     # 4 column chunks
HC = HID // 128   # 16 contraction chunks
SCALE = float(D) ** -0.5
NEG = -30000.0   # must be fp16-representable; exp(SCALE*(NEG+score)) == 0

DTYPE = "f16"     # matmul dtype: 'f16' | 'bf16' | 'f32r'


def _build(variant, dt):
    MDT = MMDT[dt]
    two_byte = dt in ("f16", "bf16")
    IDT = MDT if two_byte else F32
    nc = bacc.Bacc("TRN2", target_bir_lowering=False, debug=False,
                   num_devices=NCORES)
    xt = nc.dram_tensor("xt", [HID, S], IDT, kind="ExternalInput").ap()
    wq = nc.dram_tensor("wq", [HID, GW], IDT, kind="ExternalInput").ap()
    wk = nc.dram_tensor("wk", [HID, GW], IDT, kind="ExternalInput").ap()
    wv = nc.dram_tensor("wv", [HID, GW], IDT, kind="ExternalInput").ap()
    wo = nc.dram_tensor("wo", [GW, HID], IDT, kind="ExternalInput").ap()
    cost = nc.dram_tensor("cost", [D, S], IDT, kind="ExternalInput").ap()
    sint = nc.dram_tensor("sint", [D, S], IDT, kind="ExternalInput").ap()
    btpl = nc.dram_tensor("btpl", [D, D], IDT, kind="ExternalInput").ap()
    # fp16 output halves the out-DMA traffic; host accumulates in fp32
    ODT = MDT if two_byte else F32
    out = nc.dram_tensor("out", [S, HID], ODT, kind="ExternalOutput").ap()

    def _bc(ap):
        return ap if two_byte else ap.bitcast(F32R)

    xt_r = _bc(xt.rearrange("(c p) s -> p c s", p=128))   # [128, 16, 2048]
    wq_r = _bc(wq.rearrange("(c p) m -> p c m", p=128))   # [128, 16, 512]
    wk_r = _bc(wk.rearrange("(c p) m -> p c m", p=128))
    wv_r = _bc(wv.rearrange("(c p) m -> p c m", p=128))
    wo_r = _bc(wo.rearrange("(c p) m -> p c m", p=128))   # [128, 4, 2048]

    XB = 4                   # h-chunks per xt DMA
    NXT = HC // XB           # 4 xt tiles per s-chunk

    with tile.TileContext(nc) as tc:
        with contextlib.ExitStack() as ctx:
            persist = ctx.enter_context(tc.tile_pool(name="persist", bufs=1))
            psum = ctx.enter_context(tc.tile_pool(name="psum", bufs=1, space="PSUM"))
            work = ctx.enter_context(tc.tile_pool(name="work", bufs=1))

            _n = [0]

            def bank(i, shape=(128, 512)):
                _n[0] += 1
                return psum.tile(list(shape), F32, tag=f"b{i}", name=f"bk{i}_{_n[0]}")

            qts = [[persist.tile([128, 512], MDT, tag=f"qt{h}_{s}",
                                 name=f"qt{h}_{s}") for s in range(SC)]
                   for h in range(GH)]
            kts = [[persist.tile([128, 512], MDT, tag=f"kt{h}_{s}",
                                 name=f"kt{h}_{s}") for s in range(SC)]
                   for h in range(GH)]
            VDT = F8 if PV8 else MDT
            # v stored as k-block PAIRS [128, 2, GW]: slice [:, :, hd*128..]
            # is the [128, 2, 128] DoubleRow stationary operand
            vts2 = [persist.tile([128, 2, GW], VDT, tag=f"v{j}", name=f"v{j}")
                    for j in range(HC // 2)]
            cos_sb = persist.tile([128, S], MDT, tag="cos")
            sin_sb = persist.tile([128, S], MDT, tag="sin")
            btpl_sb = persist.tile([128, D], MDT, tag="btpl")
            wo_sb = persist.tile([128, GH, HID], MDT, tag="wo")
            # all-ones stationary operand: ones^T @ esum = partition-sum of
            # esum replicated across all 128 partitions (the softmax
            # denominator reduce, on PE instead of the 3.6us GpSimd op)
            ones_sb = persist.tile([128, D], MDT, tag="ones")

            def xt_tile(sc, j):
                t = work.tile([128, XB, 512], MDT, tag="xt", bufs=9,
                              name=f"xt_{sc}_{j}")
                if sc == 0:
                    # per-chunk pieces during the DMA ramp: subtile deps let
                    # the first consumer start when its 128KB piece lands
                    for hh in range(XB):
                        nc.sync.dma_start(
                            out=t[:, hh, :],
                            in_=xt_r[:, j * XB + hh, 0:512])
                else:
                    nc.sync.dma_start(
                        out=t, in_=xt_r[:, j * XB:(j + 1) * XB,
                                        sc * 512:(sc + 1) * 512])
                return t

            # weights for Q/K/O + tables: DMAs deferred into the A1 stream
            # (issued from the scalar engine after each sc's vts copies) so
            # the congested startup window belongs to the xt/wv stream alone.
            wq_sb = work.tile([128, HC, GW], MDT, tag="wqk", bufs=2)
            wk_sb = work.tile([128, HC, GW], MDT, tag="wqk", bufs=2)

            # ---- A1: V = X @ Wv (banks b0..b3) -------------------------
            # First j-chunk of xt/wv split per h-chunk so the first matmul
            # only waits on 2x128KB of DMA. Remaining xt tiles issue
            # interleaved with the wv chunks, kept 3 iterations ahead.
            with tc.tile_pool(name="phV", bufs=1) as phv:
                xt0 = []
                wv0 = []
                for hh in range(XB):
                    xt0_t = work.tile([128, 512], MDT, tag="xt0", bufs=XB,
                                      name=f"xt0_{hh}")
                    nc.sync.dma_start(out=xt0_t, in_=xt_r[:, hh, 0:512])
                    xt0.append(xt0_t)
                    wv0_t = phv.tile([128, GW], MDT, tag="wvf0", bufs=XB,
                                     name=f"wvf0_{hh}")
                    nc.sync.dma_start(out=wv0_t, in_=wv_r[:, hh, :])
                    wv0.append(wv0_t)
                order = [(sc, j) for sc in range(SC) for j in range(NXT)][1:]
                pend = {}
                wv_cs = [None] * NXT

                def issue(idx):
                    if idx < len(order):
                        pend[order[idx]] = xt_tile(*order[idx])
                    if idx < NXT - 1:
                        wvc = phv.tile([128, XB, GW], MDT, tag="wvf",
                                       bufs=NXT - 1, name=f"wvf_{idx + 1}")
                        nc.sync.dma_start(
                            out=wvc,
                            in_=wv_r[:, (idx + 1) * XB:(idx + 2) * XB, :])
                        wv_cs[idx + 1] = wvc

                for idx in range(3):
                    issue(idx)
                nxt_issue = 3
                # dummy matmuls on a memset tile: burn the PE clock-ramp
                # (half speed for the first ~3us of activity) and the
                # DMA-supply bubbles at the start, where PE is idle anyway.
                warm = work.tile([128, 512], MDT, tag="warm", bufs=1)
                nc.vector.memset(warm, 0.0)
                nc.vector.memset(ones_sb, 1.0)

                def warmup(n):
                    wb = bank(5)
                    for _ in range(n):
                        nc.tensor.matmul(wb, warm[:, 0:128], warm,
                                         start=True, stop=True)

                warmup(8)
                sc0_xt = {}
                for sc in range(SC):
                    # alternate bank quads per sc: the next sc's first matmul
                    # never waits on this sc's eviction copies
                    vps = [bank(st + 4 * (sc % 2)) for st in range(4)]
                    for j in range(NXT):
                        if sc == 0 and j == 1:
                            # 6 warm matmuls (~2.6us) fill most of the xt
                            # j=1 DMA-ramp stall so HAM never re-throttles
                            warmup(6)
                        if sc == 0 and j == 0:
                            xts = xt0
                            wvs = wv0
                        else:
                            xt_t = pend.pop((sc, j))
                            if sc == 0:
                                sc0_xt[j] = xt_t
                            issue(nxt_issue)
                            nxt_issue += 1
                            xts = [xt_t[:, hh, :] for hh in range(XB)]
                            wvs = ([wv0[hh] for hh in range(XB)] if j == 0
                                   else [wv_cs[j][:, hh, :] for hh in range(XB)])
                        for hh in range(XB):
                            h = j * XB + hh
                            for st in range(4):
                                nc.tensor.matmul(
                                    vps[st], xts[hh][:, st * 128:(st + 1) * 128],
                                    wvs[hh],
                                    start=(h == 0), stop=(h == HC - 1))
                                if h == HC - 1:
                                    # evict bank st while PE finishes st+1..3,
                                    # so the next sc's first matmul on bank 0
                                    # doesn't wait for the whole copy batch
                                    kb = sc * 4 + st
                                    nc.scalar.copy(
                                        vts2[kb // 2][:, kb % 2, :], vps[st])

            def sc0_get(j, hh):
                return xt0[hh] if j == 0 else sc0_xt[j][:, hh, :]

            nc.sync.dma_start(out=wq_sb, in_=wq_r)
            nc.sync.dma_start(out=wk_sb, in_=wk_r)
            nc.sync.dma_start(out=wo_sb, in_=wo_r)
            nc.sync.dma_start(out=cos_sb, in_=_bc(cost))
            nc.sync.dma_start(out=sin_sb, in_=_bc(sint))
            nc.sync.dma_start(out=btpl_sb, in_=_bc(btpl))

            # ---- A2 units: one s-chunk = Q half then K half ------------
            # reuse: (j, hh) -> AP accessor for already-resident xt tiles
            # (sc=0 reuses A1's tiles — saves 2MB of DMA in the congested
            # startup window).
            # wide=True uses 4 psum banks (one pass per half); wide=False
            # does two d-pair passes on banks {0,1} only, freeing banks 2/3
            # for the attention streams (deeper sps rotation + 2 ctx banks).
            # The pass-boundary evict latency is absorbed by the B-stream
            # units spliced between A2 units.
            def a2_units(sc, reuse=None, wide=False):
                ssl = slice(sc * 512, (sc + 1) * 512)
                units = []
                state = {}

                def prep():
                    if reuse is not None:
                        state['get'] = reuse
                    else:
                        tiles = [xt_tile(sc, j) for j in range(NXT)]
                        state['get'] = lambda j, hh: tiles[j][:, hh, :]

                units.append(prep)

                def evict(d, dsl, pss):
                    # RoPE reads the PSUM bank directly (DVE can read PSUM):
                    # no Act copy hop — frees ~22us of Act time globally so
                    # the exp stream never queues behind t2 copies; d-major
                    # ordering gives the bank 3.4us of slack vs this ~1.5us
                    # DVE chain.
                    src = pss[d]
                    t1 = work.tile([128, 512], MDT, tag="t1", bufs=3,
                                   name=f"t1_{sc}_{d}_{dsl.tensor.name}")
                    nc.vector.tensor_mul(t1, src, cos_sb[:, ssl])
                    nc.vector.tensor_mul(dsl[0:64, :], src[64:128, :],
                                         sin_sb[64:128, ssl])
                    nc.vector.tensor_mul(dsl[64:128, :], src[0:64, :],
                                         sin_sb[0:64, ssl])
                    nc.vector.tensor_add(dsl, dsl, t1)

                dgroups = ([tuple(range(GH))] if wide
                           else [(0, 1), (2, 3)])
                for half, (w_sb, dts) in enumerate(
                        ((wq_sb, [qts[d][sc] for d in range(GH)]),
                         (wk_sb, [kts[d][sc] for d in range(GH)]))):
                    for dg in dgroups:
                        pss = {}
                        # d-major: each d's 16 accumulation steps complete
                        # before the next d starts, so its evict chain (Act
                        # copy + DVE RoPE) overlaps the next d's matmuls and
                        # the pass boundary never stalls on an eviction.
                        for x, d in enumerate(dg):

                            def stepd(j, hh, d=d, x=x, w_sb=w_sb, pss=pss):
                                h = j * XB + hh
                                if h == 0:
                                    pss[d] = bank(x)
                                nc.tensor.matmul(
                                    pss[d], w_sb[:, h, d * 128:(d + 1) * 128],
                                    state['get'](j, hh),
                                    start=(h == 0), stop=(h == HC - 1))

                            for j in range(NXT):
                                for hh in range(XB):
                                    units.append(lambda j=j, hh=hh,
                                                 stepd=stepd: stepd(j, hh))
                            units.append(lambda d=d, dsl=dts[d], pss=pss:
                                         evict(d, dsl, pss))
                return units

            # ---- B units: attention for one q chunk --------------------
            # Per block: QK matmul (sps bank), optional mask add, exp, DVE
            # exp-sum accumulate, PV accumulate (ctx bank). Tail: GpSimd
            # partition_all_reduce of the exp-sum -> reciprocal -> ctx mul.
            # Returns (units, ctx_t); o_proj emitted separately.
            def b_units(qc, sps_banks, ctx_banks, sps_banks_late=None):
                if variant == "causal":
                    # (kbi, coff): off-diag first, diag last — the head's
                    # final exp/esum-add before the normalize chain is then
                    # only 128 wide, shortening the tail latency. qc=0 has
                    # no off-diag; its j=0 diag is full width and inits psum.
                    blocks = [(kb, 0) for kb in range(4 * qc)]
                    blocks += [(4 * qc + j, 128 * j) for j in range(4)]
                    noff = 4 * qc
                else:
                    blocks = [(kb, 0) for kb in range(HC)]
                    noff = HC
                nblk = len(blocks)
                ctx_t = []
                heads = []
                for hd in range(GH):
                    st = {}

                    def start_head(st=st, hd=hd):
                        st['ctxps'] = bank(ctx_banks[hd % len(ctx_banks)])
                        st['esum'] = work.tile([128, 512], MDT, tag="esum",
                                               bufs=3, name=f"esum_{qc}_{hd}")
                        st['pend'] = []

                    def flush(last, st=st, hd=hd):
                        ent = st['pend'].pop(0)
                        if ent[0] == 'dr':
                            # off-diag pair: one DoubleRow matmul covers both
                            # 128-row k-blocks of the pair tile
                            _, pex, kbp2, first = ent
                            nc.tensor.matmul(
                                st['ctxps'][:, 0:512],
                                vts2[kbp2][:, :, hd * 128:(hd + 1) * 128],
                                pex[:, :, 0:512],
                                start=first, stop=last, perf_mode=DR)
                        else:
                            _, pex, par, kbp, coff, first = ent
                            nc.tensor.matmul(
                                st['ctxps'][:, coff:512],
                                vts2[kbp // 2][:, kbp % 2,
                                               hd * 128:(hd + 1) * 128],
                                pex[:, par, coff:512],
                                start=first, stop=last)

                    sbanks = (sps_banks_late
                              if hd > 0 and sps_banks_late else sps_banks)

                    def kb_iter(i, kb, coff, st=st, hd=hd, sbanks=sbanks,
                                start_head=start_head, flush=flush):
                        if i == 0:
                            start_head()
                        sps = bank(sbanks[i % len(sbanks)])
                        diag = variant == "causal" and kb >= 4 * qc
                        nc.tensor.matmul(
                            sps[:, coff:512],
                            kts[hd][kb // 4][:, (kb % 4) * 128:(kb % 4 + 1) * 128],
                            qts[hd][qc][:, coff:512],
                            start=True, stop=True)
                        if diag:
                            nc.vector.tensor_add(sps[:, coff:coff + 128],
                                                 sps[:, coff:coff + 128],
                                                 btpl_sb)
                        par = i % 2
                        if par == 0:
                            st['pex'] = work.tile([128, 2, 512], VDT,
                                                  tag="pexp", bufs=6,
                                                  name=f"pexp_{qc}_{hd}_{kb}")
                        pex = st['pex']
                        nc.scalar.activation(pex[:, par, coff:512],
                                             sps[:, coff:512], EXP, scale=SCALE)
                        if i == 0:
                            nc.vector.tensor_copy(st['esum'], pex[:, 0, :])
                        else:
                            nc.vector.tensor_add(st['esum'][:, coff:512],
                                                 st['esum'][:, coff:512],
                                                 pex[:, par, coff:512])
                        ent = None
                        if PV8 and DR_ON and i < noff:
                            if par == 1:
                                ent = ('dr', pex, kb // 2, i == 1)
                        else:
                            ent = ('sg', pex, par, kb, coff, i == 0)
                        if ent is not None:
                            if len(st['pend']) >= 4:
                                flush(False)
                            st['pend'].append(ent)

                    def tail_flush(st=st, flush=flush):
                        while len(st['pend']) > 1:
                            flush(False)
                        flush(True)

                    def tail_norm(st=st, hd=hd, sbanks=sbanks):
                        # denominator: ones^T @ esum sums the 128 partitions,
                        # result replicated to every partition of a psum bank.
                        # Bank choice: latest-used slot of the next head's
                        # rotation that this head has already freed, so the
                        # den matmul neither waits on the next head's exp nor
                        # blocks its early QK matmuls.
                        di = len(sbanks) - 1
                        if (nblk - 1) % len(sbanks) == di:
                            di -= 1
                        dps = bank(sbanks[di])
                        nc.tensor.matmul(dps, ones_sb, st['esum'],
                                         start=True, stop=True)
                        dbc = work.tile([128, 512], F32, tag="dbc", bufs=2,
                                        name=f"dbc_{qc}_{hd}")
                        # den in [1, ~4e3]: far from approx_fast edge cases
                        nc.vector.reciprocal_approx_fast(dbc, dps)
                        ct = work.tile([128, 512], MDT, tag="ctx", bufs=9,
                                       name=f"ctx_{qc}_{hd}")
                        nc.vector.tensor_mul(ct, st['ctxps'], dbc)  # frees bank
                        ctx_t.append(ct)

                    hu = [lambda i=i, kb=kb, coff=coff, kb_iter=kb_iter:
                          kb_iter(i, kb, coff)
                          for i, (kb, coff) in enumerate(blocks)]
                    hu.append(tail_flush)
                    heads.append((hu, tail_norm))
                # weave: head h's den/normalize chain is emitted after head
                # h+1's first two QK units, so the in-order PE absorbs the
                # esum-chain wait with useful matmuls.
                units = []
                prev_norm = None
                for hu, tnorm in heads:
                    units += hu[:2]
                    if prev_norm is not None:
                        units.append(prev_norm)
                    units += hu[2:]
                    prev_norm = tnorm
                units.append(prev_norm)
                return units, ctx_t

            # ---- o_proj units: stationary-reuse order, grouped banks ---
            # pipeline>1 runs that many accumulation groups (1 ob each, own
            # bank) in a wavefront, so each group's hd=3 stop-matmul — the
            # one gated on the last head's normalize chain — sits several
            # units later in PE program order.
            def oproj_units(qc, ctx_t, banks, pipeline=1, evict_eng='alt',
                            taper_halves=False, dma_scalar=False,
                            wave_dma=False):
                units = []
                ng = len(banks)
                if pipeline > 1:
                    groups = [(qb, [ob]) for qb in range(4) for ob in range(4)]
                    nob = 1
                else:
                    groups = [(qb, list(range(og, og + ng)))
                              for qb in range(4) for og in range(0, 4, ng)]
                    nob = ng

                def mm(hd, qb, obs, st2, b0):
                    if hd == 0:
                        st2['ops'] = {ob: bank(banks[(b0 + x) % ng])
                                      for x, ob in enumerate(obs)}
                    for ob in obs:
                        nc.tensor.matmul(
                            st2['ops'][ob],
                            ctx_t[hd][:, qb * 128:(qb + 1) * 128],
                            wo_sb[:, hd, ob * 512:(ob + 1) * 512],
                            start=(hd == 0), stop=(hd == GH - 1))

                def evict(qc_, qb, obs, st2, halves=False, force_vec=False):
                    if wave_dma and len(obs) == 4:
                        # pipeline=1 group = one full row-block: stage the 4
                        # banks into one contiguous tile, ONE output DMA
                        # (saves 3 x ~0.6us sync issue slots that would
                        # otherwise contend with the A2 xt-supply DMAs)
                        rows = slice((qc_ * 4 + qb) * 128,
                                     (qc_ * 4 + qb + 1) * 128)
                        wt = work.tile([128, 2048], ODT, tag="wave", bufs=2,
                                       name=f"wave_{qc_}_{qb}")
                        for ob in obs:
                            eng = (nc.scalar.copy if ob % 2 == 0
                                   else nc.vector.tensor_copy)
                            eng(wt[:, ob * 512:(ob + 1) * 512], st2['ops'][ob])
                        nc.sync.dma_start(out=out[rows, :], in_=wt)
                        return
                    for x, ob in enumerate(obs):
                        if evict_eng == 'vector' or force_vec:
                            eng = nc.vector.tensor_copy
                        elif (qb + ob) % 2 == 0:
                            eng = nc.scalar.copy
                        else:
                            eng = nc.vector.tensor_copy
                        rows = slice((qc_ * 4 + qb) * 128,
                                     (qc_ * 4 + qb + 1) * 128)
                        if halves:
                            # separate half-tiles so Act and DVE copy
                            # concurrently; both copies emitted before the
                            # DMAs; DMAs on separate issue queues
                            oths = []
                            for qi, h0 in enumerate((0, 256)):
                                oth = work.tile([128, 256], ODT, tag="outh",
                                                bufs=2,
                                                name=f"oth_{qc_}_{qb}_{ob}_{qi}")
                                hs = slice(h0, h0 + 256)
                                ceng = (nc.scalar.copy if qi == 0
                                        else nc.vector.tensor_copy)
                                ceng(oth, st2['ops'][ob][:, hs])
                                oths.append(oth)
                            for qi, h0 in enumerate((0, 256)):
                                # final halves on two separate issue queues
                                deng = nc.scalar if qi == 0 else nc.sync
                                deng.dma_start(
                                    out=out[rows,
                                            ob * 512 + h0:ob * 512 + h0 + 256],
                                    in_=oths[qi])
                        else:
                            ot = work.tile([128, 512], ODT, tag="outsb",
                                           bufs=4, name=f"ot_{qc_}_{qb}_{ob}")
                            eng(ot, st2['ops'][ob])
                            deng = (nc.scalar if dma_scalar
                                    and eng is nc.scalar.copy else nc.sync)
                            deng.dma_start(
                                out=out[rows, ob * 512:(ob + 1) * 512],
                                in_=ot)

                def evict_wave(qc_, chunk, sts, last):
                    # pipeline=4 wave = one qb row-block x all 4 obs: stage
                    # the 4 psum banks into one contiguous [128,2048] tile
                    # (copies alternate Act/DVE) and DMA it in ONE transfer
                    # (4KB/partition descriptors, 1 issue instead of 4). The
                    # final wave splits across two issue queues so its
                    # completion isn't gated on one 1MB transfer.
                    qb = chunk[0][0]
                    rows = slice((qc_ * 4 + qb) * 128, (qc_ * 4 + qb + 1) * 128)
                    wt = work.tile([128, 2048], ODT, tag="wave", bufs=2,
                                   name=f"wave_{qc_}_{qb}")
                    for (qb_, obs), (st2, b0) in zip(chunk, sts):
                        ob = obs[0]
                        eng = (nc.scalar.copy if ob % 2 == 0
                               else nc.vector.tensor_copy)
                        eng(wt[:, ob * 512:(ob + 1) * 512], st2['ops'][ob])
                    if last:
                        nc.scalar.dma_start(out=out[rows, 0:1024],
                                            in_=wt[:, 0:1024])
                        nc.sync.dma_start(out=out[rows, 1024:2048],
                                          in_=wt[:, 1024:2048])
                    else:
                        nc.sync.dma_start(out=out[rows, :], in_=wt)

                nchunk = (len(groups) + pipeline - 1) // pipeline
                for ci, c0 in enumerate(range(0, len(groups), pipeline)):
                    chunk = groups[c0:c0 + pipeline]
                    sts = [({}, (c0 + gi) * nob) for gi in range(len(chunk))]
                    last_chunk = ci == nchunk - 1
                    for hd in range(GH):
                        for (qb, obs), (st2, b0) in zip(chunk, sts):
                            units.append(
                                lambda hd=hd, qb=qb, obs=obs, st2=st2, b0=b0:
                                mm(hd, qb, obs, st2, b0))
                    if wave_dma:
                        units.append(lambda chunk=chunk, sts=sts,
                                     lc=last_chunk:
                                     evict_wave(qc, chunk, sts, lc))
                        continue
                    for (qb, obs), (st2, b0) in zip(chunk, sts):
                        hv = taper_halves and last_chunk
                        # second-to-last chunk evicts on DVE so Act is idle
                        # when the final halves need it
                        fv = taper_halves and ci == nchunk - 2
                        units.append(lambda qb=qb, obs=obs, st2=st2, hv=hv,
                                     fv=fv:
                                     evict(qc, qb, obs, st2, halves=hv,
                                           force_vec=fv))
                return units

            # ---- emit ---------------------------------------------------
            def splice(main, filler, lead=0):
                # lead: number of main units to run before filler starts
                na, nb = len(filler), len(main)
                ai = 0
                for i, u in enumerate(main):
                    u()
                    eff = max(0, i + 1 - lead)
                    den = max(1, nb - lead)
                    tgt = min(na, eff * na // den) if nb else na
                    while ai < tgt:
                        filler[ai]()
                        ai += 1
                while ai < na:
                    filler[ai]()
                    ai += 1

            ctxs = {}
            if variant != "causal":
                # full attention: every q chunk needs all K chunks, so all
                # projections must complete before the attention stream.
                for sc in range(SC):
                    for u in a2_units(sc, reuse=sc0_get if sc == 0 else None,
                                      wide=True):
                        u()
                for qc in range(SC):
                    bu, ctxs[qc] = b_units(qc, sps_banks=(4, 5, 7),
                                           ctx_banks=(6,))
                    bu += oproj_units(qc, ctxs[qc], banks=(0, 1, 2, 3))
                    for u in bu:
                        u()
            else:
                for u in a2_units(0, reuse=sc0_get, wide=True):
                    u()
                for qc in range(SC - 1):
                    # A2 filler only holds banks 0/1 now: banks 2 and 3 are
                    # free for a 4-deep sps rotation and a second ctx bank.
                    bu, ctxs[qc] = b_units(qc, sps_banks=(4, 5, 7, 2),
                                           ctx_banks=(6, 3))
                    if qc < 2:
                        bu += oproj_units(qc, ctxs[qc], banks=(4, 5, 6, 7))
                    splice(bu, a2_units(qc + 1))
                # qc=3: no A2 filler; defer qc=2's o_proj into this stream,
                # software-pipelined so each group's ct[3]-gated stop-matmul
                # sits late in PE program order. sps must avoid bank 6
                # (qc2's ctx bank: its normalize chain drains late and would
                # stall the in-order PE on the WAR dep); ctx on banks 2/3
                # (A2 pss banks, free since mid-qc2).
                bu, ctxs[3] = b_units(3, sps_banks=(4, 5, 7), ctx_banks=(2, 3),
                                      sps_banks_late=(4, 5, 7, 6))
                op2 = oproj_units(2, ctxs[2], banks=(0, 1), pipeline=2,
                                  evict_eng='alt')
                splice(bu, op2[:-6])
                for u in op2[-6:]:
                    u()
                # all non-ctx banks are drained by now; pipeline=4 makes each
                # wave one full row-block (qb x all obs) so it evicts into a
                # contiguous staging tile and DMAs once per wave.
                for u in oproj_units(3, ctxs[3], banks=(0, 1, 6, 4, 5, 7),
                                     pipeline=4, wave_dma=True):
                    u()
    nc.compile()
    return nc


_CACHE = {}


def _get(variant, dt=None):
    dt = dt or DTYPE
    if (variant, dt) not in _CACHE:
        _CACHE[(variant, dt)] = _build(variant, dt)
    return _CACHE[(variant, dt)]


def _rope_tables():
    inv = 1.0 / (10000.0 ** (np.arange(0, D, 2, dtype=np.float64) / D))  # [64]
    t = np.arange(S, dtype=np.float64)
    fr = np.outer(inv, t)                       # [64, S]
    cosT = np.concatenate([np.cos(fr), np.cos(fr)], 0).astype(np.float32)
    # partition-swapped sign-folded sin: rows 0:64 = +sin, rows 64:128 = -sin
    sinT = np.concatenate([np.sin(fr), -np.sin(fr)], 0).astype(np.float32)
    return cosT, sinT


def _btpl_causal():
    # additive triangle mask template: NEG where k > c (128x128)
    k = np.arange(128)[:, None]
    c = np.arange(128)[None, :]
    return np.where(k > c, np.float32(NEG), np.float32(0.0)).astype(np.float32)


def _np_cast(a, dt):
    if dt == "f16":
        return a.astype(np.float16)
    if dt == "bf16":
        import ml_dtypes
        return a.astype(ml_dtypes.bfloat16)
    return a


def _numpy_fallback(hs, Wq, Wk, Wv, Wo, mask):
    B = hs.shape[0]
    cosT, sinT = _rope_tables()
    cos = cosT.T[None, :, None, :]
    sin = np.abs(sinT).T[None, :, None, :]
    outs = []
    for b in range(B):
        x = hs[b]
        q = (x @ Wq).reshape(S, 16, D)[None]
        k = (x @ Wk).reshape(S, 16, D)[None]
        vv = (x @ Wv).reshape(S, 16, D)

        def rope(z):
            z1, z2 = z[..., :64], z[..., 64:]
            rot = np.concatenate([-z2, z1], -1)
            return z * cos + rot * sin

        q, k = rope(q)[0], rope(k)[0]
        o = np.empty((S, 16, D), np.float32)
        m = mask[0, 0]
        for h in range(16):
            sc = (q[:, h] @ k[:, h].T) * SCALE
            sc = np.where(m == 0, -np.inf, sc)
            sc -= sc.max(-1, keepdims=True)
            p = np.exp(sc)
            p /= p.sum(-1, keepdims=True)
            o[:, h] = p @ vv[:, h]
        outs.append(o.reshape(S, HID) @ Wo)
    return np.stack(outs).astype(np.float32)


def kernel(hidden_states, Wq, Wk, Wv, Wo, attention_mask):
    hs = np.asarray(hidden_states, dtype=np.float32)
    Wq, Wk, Wv, Wo = (np.asarray(w, dtype=np.float32) for w in (Wq, Wk, Wv, Wo))
    mask = np.asarray(attention_mask)
    B = hs.shape[0]

    m3 = mask.reshape(-1, mask.shape[-2], mask.shape[-1])
    m2 = m3[0]
    same = all(np.array_equal(m2, m3[i]) for i in range(1, m3.shape[0]))
    if not same:
        return _numpy_fallback(hs, Wq, Wk, Wv, Wo, mask)
    if np.all(m2 == 1):
        variant = "full"
    elif np.array_equal(m2 != 0, np.tril(np.ones((S, S), dtype=bool))):
        variant = "causal"
    else:
        return _numpy_fallback(hs, Wq, Wk, Wv, Wo, mask)

    cosT, sinT = _rope_tables()
    btpl = _btpl_causal() if variant == "causal" else np.zeros((128, 128), np.float32)

    in_maps = []
    for c in range(NCORES):
        b, g = divmod(c, GH)
        gsl = slice(g * GW, (g + 1) * GW)
        in_maps.append({
            "xt": _np_cast(np.ascontiguousarray(hs[b].T), DTYPE),
            "wq": _np_cast(np.ascontiguousarray(Wq[:, gsl]), DTYPE),
            "wk": _np_cast(np.ascontiguousarray(Wk[:, gsl]), DTYPE),
            "wv": _np_cast(np.ascontiguousarray(Wv[:, gsl]), DTYPE),
            "wo": _np_cast(np.ascontiguousarray(Wo[gsl, :]), DTYPE),
            "cost": _np_cast(cosT, DTYPE), "sint": _np_cast(sinT, DTYPE),
            "btpl": _np_cast(btpl, DTYPE),
        })

    nc = _get(variant)
    res = run_bass_kernel_spmd(nc, in_maps, list(range(NCORES))).results
    out = np.zeros((B, S, HID), np.float32)
    for c in range(NCORES):
        b = c // GH
        out[b] += res[c]["out"]
    return out



# revision 39
# speedup vs baseline: 1.0038x; 1.0038x over previous
"""Trainium2 Bass kernel for nn_Attention (dense transformer attention block).

Full-input contract: kernel(**inputs) takes the unsharded inputs and returns
the full output. 8 NeuronCores: tensor-parallel over head groups (4 heads) x
data-parallel over batch (2); core c = b*4 + g. Per core: q/k/v projections
for its head group, RoPE, causal flash-style attention (transposed-P layout,
softmax without max-subtraction), partial o_proj with its rows of Wo; the 4
partials per batch element are summed on the host (the all-reduce of the
row-sharded o_proj).

Optimizations over the first working version (sim 351us -> 287us; HW
366us -> 317us at 2.4GHz):
 - softmax denominator via DVE exp-sum accumulation + ONE ones-matmul
   partition reduce on PE per (head, q-chunk) (the GpSimd
   partition_all_reduce used before costs 3.6us per call on HW and
   serialized the normalize chains: replacing it won -47us). Each head's
   den/recip/ctx-mul chain is woven after the next head's first two QK
   units so the in-order PE absorbs the esum-chain wait.
 - A1 alternates psum bank quads per sc; A2 is d-major (evict chains
   overlap the next d's matmuls); sc0 xt DMAs split per 128KB piece.
 - NOTE: fp8e4m3 PV (DoubleRow) was tried and REJECTED: fp8 noise on
   v/pexp does not average down (signal shrinks with the same 1/sqrt(n)
   as the noise); measured ~3.5% output error vs the 2% budget.
 - NOTE: HW runs sometimes execute with PE at 2.0GHz (P0 power state)
   instead of 2.4GHz -- wall times inflate ~20% run-to-run; check the
   median matmul duration (379ns warm @2.4 vs 454ns @2.0) to compare.
 - causal diagonal blocks trimmed to their valid column range (QK, exp,
   mask add, PV all operate on [128j:512) only); diag blocks ordered last
   so each head's tail exp/esum chain is short.
 - deep PV pend pipeline (8 blocks): exp-gated PV matmuls never stall the
   in-order PE mid-stream; they drain as pure-PE bursts.
 - A2 projection split into two d-pair passes on psum banks {0,1} only
   (same cycles), freeing banks 2/3 for a 4-deep sps rotation and a second
   ctx bank in the attention streams.
 - ctx normalization multiply reads the PSUM bank directly (no scalar
   copy); reciprocal via the fast approx DVE op.
 - o_proj software-pipelined over grouped PSUM banks with stationary
   reuse; qc=2's o_proj deferred into the qc=3 stream as PE filler (qc=3
   has no projection filler and is exp-latency-bound); final group's
   evict+DMA split in halves across Act/DVE copies and both DMA issue
   queues to shorten the end drain.
 - first xt/wv DMAs split into 128KB pieces, xt prefetch pipelined, sc=0
   xt tiles reused by A2(0); fp16 output (host accumulates in fp32).
 - dummy warmup matmuls burn the PE clock-ramp during the DMA-idle start.

The Q/K projection work (phase A2) is interleaved into the attention stream
(phase B) as PE filler: B-qc only needs Q/K columns of chunks <= qc, so
A2-sc(qc+1) runs alongside B-qc, hiding the exp/softmax latency chains.

Matmul dtype configurable (DTYPE): fp16 default (~7e-4 rel err), f32r
fallback (~3.5e-4).
"""
import contextlib
import numpy as np
import concourse.bass as bass
from concourse import bacc, bass_isa
import concourse.mybir as mybir
import concourse.tile as tile
from concourse.bass_utils import run_bass_kernel_spmd

F32 = mybir.dt.float32
F32R = mybir.dt.float32r
F16 = mybir.dt.float16
BF16 = mybir.dt.bfloat16
F8 = mybir.dt.float8e4
DR = mybir.MatmulPerfMode.DoubleRow
EXP = mybir.ActivationFunctionType.Exp
MMDT = {"f32r": F32R, "f16": F16, "bf16": BF16}
# fp8 PV (DoubleRow) was tried and REJECTED: per-element fp8e4m3 noise on
# v/pexp does not average down (signal shrinks with the same 1/sqrt(n) as
# the noise) -- measured ~3.5%% output error vs the 2%% budget. Keep f16.
PV8 = False
DR_ON = False

S = 2048
HID = 2048
D = 128
GH = 4            # heads per core
GW = GH * D       # 512
NCORES = 8
SC = S // 512     # 4 column chunks
HC = HID // 128   # 16 contraction chunks
SCALE = float(D) ** -0.5
NEG = -30000.0   # must be fp16-representable; exp(SCALE*(NEG+score)) == 0

DTYPE = "f16"     # matmul dtype: 'f16' | 'bf16' | 'f32r'


def _build(variant, dt):
    MDT = MMDT[dt]
    two_byte = dt in ("f16", "bf16")
    IDT = MDT if two_byte else F32
    nc = bacc.Bacc("TRN2", target_bir_lowering=False, debug=False,
                   num_devices=NCORES)
    xt = nc.dram_tensor("xt", [HID, S], IDT, kind="ExternalInput").ap()
    wq = nc.dram_tensor("wq", [HID, GW], IDT, kind="ExternalInput").ap()
    wk = nc.dram_tensor("wk", [HID, GW], IDT, kind="ExternalInput").ap()
    wv = nc.dram_tensor("wv", [HID, GW], IDT, kind="ExternalInput").ap()
    wo = nc.dram_tensor("wo", [GW, HID], IDT, kind="ExternalInput").ap()
    cost = nc.dram_tensor("cost", [D, S], IDT, kind="ExternalInput").ap()
    sint = nc.dram_tensor("sint", [D, S], IDT, kind="ExternalInput").ap()
    btpl = nc.dram_tensor("btpl", [D, D], IDT, kind="ExternalInput").ap()
    # fp16 output halves the out-DMA traffic; host accumulates in fp32
    ODT = MDT if two_byte else F32
    out = nc.dram_tensor("out", [S, HID], ODT, kind="ExternalOutput").ap()

    def _bc(ap):
        return ap if two_byte else ap.bitcast(F32R)

    xt_r = _bc(xt.rearrange("(c p) s -> p c s", p=128))   # [128, 16, 2048]
    wq_r = _bc(wq.rearrange("(c p) m -> p c m", p=128))   # [128, 16, 512]
    wk_r = _bc(wk.rearrange("(c p) m -> p c m", p=128))
    wv_r = _bc(wv.rearrange("(c p) m -> p c m", p=128))
    wo_r = _bc(wo.rearrange("(c p) m -> p c m", p=128))   # [128, 4, 2048]

    XB = 4                   # h-chunks per xt DMA
    NXT = HC // XB           # 4 xt tiles per s-chunk

    with tile.TileContext(nc) as tc:
        with contextlib.ExitStack() as ctx:
            persist = ctx.enter_context(tc.tile_pool(name="persist", bufs=1))
            psum = ctx.enter_context(tc.tile_pool(name="psum", bufs=1, space="PSUM"))
            work = ctx.enter_context(tc.tile_pool(name="work", bufs=1))

            _n = [0]

            def bank(i, shape=(128, 512)):
                _n[0] += 1
                return psum.tile(list(shape), F32, tag=f"b{i}", name=f"bk{i}_{_n[0]}")

            qts = [[persist.tile([128, 512], MDT, tag=f"qt{h}_{s}",
                                 name=f"qt{h}_{s}") for s in range(SC)]
                   for h in range(GH)]
            kts = [[persist.tile([128, 512], MDT, tag=f"kt{h}_{s}",
                                 name=f"kt{h}_{s}") for s in range(SC)]
                   for h in range(GH)]
            VDT = F8 if PV8 else MDT
            # v stored as k-block PAIRS [128, 2, GW]: slice [:, :, hd*128..]
            # is the [128, 2, 128] DoubleRow stationary operand
            vts2 = [persist.tile([128, 2, GW], VDT, tag=f"v{j}", name=f"v{j}")
                    for j in range(HC // 2)]
            cos_sb = persist.tile([128, S], MDT, tag="cos")
            sin_sb = persist.tile([128, S], MDT, tag="sin")
            btpl_sb = persist.tile([128, D], MDT, tag="btpl")
            wo_sb = persist.tile([128, GH, HID], MDT, tag="wo")
            # all-ones stationary operand: ones^T @ esum = partition-sum of
            # esum replicated across all 128 partitions (the softmax
            # denominator reduce, on PE instead of the 3.6us GpSimd op)
            ones_sb = persist.tile([128, D], MDT, tag="ones")

            def xt_tile(sc, j):
                t = work.tile([128, XB, 512], MDT, tag="xt", bufs=9,
                              name=f"xt_{sc}_{j}")
                if sc == 0:
                    # per-chunk pieces during the DMA ramp: subtile deps let
                    # the first consumer start when its 128KB piece lands
                    for hh in range(XB):
                        nc.sync.dma_start(
                            out=t[:, hh, :],
                            in_=xt_r[:, j * XB + hh, 0:512])
                else:
                    nc.sync.dma_start(
                        out=t, in_=xt_r[:, j * XB:(j + 1) * XB,
                                        sc * 512:(sc + 1) * 512])
                return t

            # weights for Q/K/O + tables: DMAs deferred into the A1 stream
            # (issued from the scalar engine after each sc's vts copies) so
            # the congested startup window belongs to the xt/wv stream alone.
            wq_sb = work.tile([128, HC, GW], MDT, tag="wqk", bufs=2)
            wk_sb = work.tile([128, HC, GW], MDT, tag="wqk", bufs=2)

            # ---- A1: V = X @ Wv (banks b0..b3) -------------------------
            # First j-chunk of xt/wv split per h-chunk so the first matmul
            # only waits on 2x128KB of DMA. Remaining xt tiles issue
            # interleaved with the wv chunks, kept 3 iterations ahead.
            with tc.tile_pool(name="phV", bufs=1) as phv:
                xt0 = []
                wv0 = []
                for hh in range(XB):
                    xt0_t = work.tile([128, 512], MDT, tag="xt0", bufs=XB,
                                      name=f"xt0_{hh}")
                    nc.sync.dma_start(out=xt0_t, in_=xt_r[:, hh, 0:512])
                    xt0.append(xt0_t)
                    wv0_t = phv.tile([128, GW], MDT, tag="wvf0", bufs=XB,
                                     name=f"wvf0_{hh}")
                    nc.sync.dma_start(out=wv0_t, in_=wv_r[:, hh, :])
                    wv0.append(wv0_t)
                order = [(sc, j) for sc in range(SC) for j in range(NXT)][1:]
                pend = {}
                wv_cs = [None] * NXT

                def issue(idx):
                    if idx < len(order):
                        pend[order[idx]] = xt_tile(*order[idx])
                    if idx < NXT - 1:
                        wvc = phv.tile([128, XB, GW], MDT, tag="wvf",
                                       bufs=NXT - 1, name=f"wvf_{idx + 1}")
                        nc.sync.dma_start(
                            out=wvc,
                            in_=wv_r[:, (idx + 1) * XB:(idx + 2) * XB, :])
                        wv_cs[idx + 1] = wvc

                for idx in range(3):
                    issue(idx)
                nxt_issue = 3
                # dummy matmuls on a memset tile: burn the PE clock-ramp
                # (half speed for the first ~3us of activity) and the
                # DMA-supply bubbles at the start, where PE is idle anyway.
                warm = work.tile([128, 512], MDT, tag="warm", bufs=1)
                nc.vector.memset(warm, 0.0)
                nc.vector.memset(ones_sb, 1.0)

                def warmup(n):
                    wb = bank(5)
                    for _ in range(n):
                        nc.tensor.matmul(wb, warm[:, 0:128], warm,
                                         start=True, stop=True)

                warmup(8)
                sc0_xt = {}
                for sc in range(SC):
                    # alternate bank quads per sc: the next sc's first matmul
                    # never waits on this sc's eviction copies
                    vps = [bank(st + 4 * (sc % 2)) for st in range(4)]
                    for j in range(NXT):
                        if sc == 0 and j == 1:
                            # 6 warm matmuls (~2.6us) fill most of the xt
                            # j=1 DMA-ramp stall so HAM never re-throttles
                            warmup(6)
                        if sc == 0 and j == 0:
                            xts = xt0
                            wvs = wv0
                        else:
                            xt_t = pend.pop((sc, j))
                            if sc == 0:
                                sc0_xt[j] = xt_t
                            issue(nxt_issue)
                            nxt_issue += 1
                            xts = [xt_t[:, hh, :] for hh in range(XB)]
                            wvs = ([wv0[hh] for hh in range(XB)] if j == 0
                                   else [wv_cs[j][:, hh, :] for hh in range(XB)])
                        for hh in range(XB):
                            h = j * XB + hh
                            for st in range(4):
                                nc.tensor.matmul(
                                    vps[st], xts[hh][:, st * 128:(st + 1) * 128],
                                    wvs[hh],
                                    start=(h == 0), stop=(h == HC - 1))
                                if h == HC - 1:
                                    # evict bank st while PE finishes st+1..3,
                                    # so the next sc's first matmul on bank 0
                                    # doesn't wait for the whole copy batch
                                    kb = sc * 4 + st
                                    nc.scalar.copy(
                                        vts2[kb // 2][:, kb % 2, :], vps[st])

            def sc0_get(j, hh):
                return xt0[hh] if j == 0 else sc0_xt[j][:, hh, :]

            nc.sync.dma_start(out=wq_sb, in_=wq_r)
            nc.sync.dma_start(out=wk_sb, in_=wk_r)
            nc.sync.dma_start(out=wo_sb, in_=wo_r)
            nc.sync.dma_start(out=cos_sb, in_=_bc(cost))
            nc.sync.dma_start(out=sin_sb, in_=_bc(sint))
            nc.sync.dma_start(out=btpl_sb, in_=_bc(btpl))

            # ---- A2 units: one s-chunk = Q half then K half ------------
            # reuse: (j, hh) -> AP accessor for already-resident xt tiles
            # (sc=0 reuses A1's tiles — saves 2MB of DMA in the congested
            # startup window).
            # wide=True uses 4 psum banks (one pass per half); wide=False
            # does two d-pair passes on banks {0,1} only, freeing banks 2/3
            # for the attention streams (deeper sps rotation + 2 ctx banks).
            # The pass-boundary evict latency is absorbed by the B-stream
            # units spliced between A2 units.
            def a2_units(sc, reuse=None, wide=False):
                ssl = slice(sc * 512, (sc + 1) * 512)
                units = []
                state = {}

                def prep():
                    if reuse is not None:
                        state['get'] = reuse
                    else:
                        tiles = [xt_tile(sc, j) for j in range(NXT)]
                        state['get'] = lambda j, hh: tiles[j][:, hh, :]

                units.append(prep)

                def evict(d, dsl, pss):
                    # RoPE reads the PSUM bank directly (DVE can read PSUM):
                    # no Act copy hop — frees ~22us of Act time globally so
                    # the exp stream never queues behind t2 copies; d-major
                    # ordering gives the bank 3.4us of slack vs this ~1.5us
                    # DVE chain.
                    src = pss[d]
                    t1 = work.tile([128, 512], MDT, tag="t1", bufs=3,
                                   name=f"t1_{sc}_{d}_{dsl.tensor.name}")
                    nc.vector.tensor_mul(t1, src, cos_sb[:, ssl])
                    nc.vector.tensor_mul(dsl[0:64, :], src[64:128, :],
                                         sin_sb[64:128, ssl])
                    nc.vector.tensor_mul(dsl[64:128, :], src[0:64, :],
                                         sin_sb[0:64, ssl])
                    nc.vector.tensor_add(dsl, dsl, t1)

                dgroups = ([tuple(range(GH))] if wide
                           else [(0, 1), (2, 3)])
                for half, (w_sb, dts) in enumerate(
                        ((wq_sb, [qts[d][sc] for d in range(GH)]),
                         (wk_sb, [kts[d][sc] for d in range(GH)]))):
                    for dg in dgroups:
                        pss = {}
                        # d-major: each d's 16 accumulation steps complete
                        # before the next d starts, so its evict chain (Act
                        # copy + DVE RoPE) overlaps the next d's matmuls and
                        # the pass boundary never stalls on an eviction.
                        for x, d in enumerate(dg):

                            def stepd(j, hh, d=d, x=x, w_sb=w_sb, pss=pss):
                                h = j * XB + hh
                                if h == 0:
                                    pss[d] = bank(x)
                                nc.tensor.matmul(
                                    pss[d], w_sb[:, h, d * 128:(d + 1) * 128],
                                    state['get'](j, hh),
                                    start=(h == 0), stop=(h == HC - 1))

                            for j in range(NXT):
                                for hh in range(XB):
                                    units.append(lambda j=j, hh=hh,
                                                 stepd=stepd: stepd(j, hh))
                            units.append(lambda d=d, dsl=dts[d], pss=pss:
                                         evict(d, dsl, pss))
                return units

            # ---- B units: attention for one q chunk --------------------
            # Per block: QK matmul (sps bank), optional mask add, exp, DVE
            # exp-sum accumulate, PV accumulate (ctx bank). Tail: GpSimd
            # partition_all_reduce of the exp-sum -> reciprocal -> ctx mul.
            # Returns (units, ctx_t); o_proj emitted separately.
            def b_units(qc, sps_banks, ctx_banks, sps_banks_late=None):
                if variant == "causal":
                    # (kbi, coff): off-diag first, diag last — the head's
                    # final exp/esum-add before the normalize chain is then
                    # only 128 wide, shortening the tail latency. qc=0 has
                    # no off-diag; its j=0 diag is full width and inits psum.
                    blocks = [(kb, 0) for kb in range(4 * qc)]
                    blocks += [(4 * qc + j, 128 * j) for j in range(4)]
                    noff = 4 * qc
                else:
                    blocks = [(kb, 0) for kb in range(HC)]
                    noff = HC
                nblk = len(blocks)
                ctx_t = []
                heads = []
                for hd in range(GH):
                    st = {}

                    def start_head(st=st, hd=hd):
                        st['ctxps'] = bank(ctx_banks[hd % len(ctx_banks)])
                        st['esum'] = work.tile([128, 512], MDT, tag="esum",
                                               bufs=3, name=f"esum_{qc}_{hd}")
                        st['pend'] = []

                    def flush(last, st=st, hd=hd):
                        ent = st['pend'].pop(0)
                        if ent[0] == 'dr':
                            # off-diag pair: one DoubleRow matmul covers both
                            # 128-row k-blocks of the pair tile
                            _, pex, kbp2, first = ent
                            nc.tensor.matmul(
                                st['ctxps'][:, 0:512],
                                vts2[kbp2][:, :, hd * 128:(hd + 1) * 128],
                                pex[:, :, 0:512],
                                start=first, stop=last, perf_mode=DR)
                        else:
                            _, pex, par, kbp, coff, first = ent
                            nc.tensor.matmul(
                                st['ctxps'][:, coff:512],
                                vts2[kbp // 2][:, kbp % 2,
                                               hd * 128:(hd + 1) * 128],
                                pex[:, par, coff:512],
                                start=first, stop=last)

                    sbanks = (sps_banks_late
                              if hd > 0 and sps_banks_late else sps_banks)

                    def kb_iter(i, kb, coff, st=st, hd=hd, sbanks=sbanks,
                                start_head=start_head, flush=flush):
                        if i == 0:
                            start_head()
                        sps = bank(sbanks[i % len(sbanks)])
                        diag = variant == "causal" and kb >= 4 * qc
                        nc.tensor.matmul(
                            sps[:, coff:512],
                            kts[hd][kb // 4][:, (kb % 4) * 128:(kb % 4 + 1) * 128],
                            qts[hd][qc][:, coff:512],
                            start=True, stop=True)
                        if diag:
                            nc.vector.tensor_add(sps[:, coff:coff + 128],
                                                 sps[:, coff:coff + 128],
                                                 btpl_sb)
                        par = i % 2
                        if par == 0:
                            st['pex'] = work.tile([128, 2, 512], VDT,
                                                  tag="pexp", bufs=6,
                                                  name=f"pexp_{qc}_{hd}_{kb}")
                        pex = st['pex']
                        nc.scalar.activation(pex[:, par, coff:512],
                                             sps[:, coff:512], EXP, scale=SCALE)
                        if i == 0:
                            nc.vector.tensor_copy(st['esum'], pex[:, 0, :])
                        else:
                            nc.vector.tensor_add(st['esum'][:, coff:512],
                                                 st['esum'][:, coff:512],
                                                 pex[:, par, coff:512])
                        ent = None
                        if PV8 and DR_ON and i < noff:
                            if par == 1:
                                ent = ('dr', pex, kb // 2, i == 1)
                        else:
                            ent = ('sg', pex, par, kb, coff, i == 0)
                        if ent is not None:
                            if len(st['pend']) >= 4:
                                flush(False)
                            st['pend'].append(ent)

                    def tail_flush(st=st, flush=flush):
                        while len(st['pend']) > 1:
                            flush(False)
                        flush(True)

                    def tail_norm(st=st, hd=hd, sbanks=sbanks):
                        # denominator: ones^T @ esum sums the 128 partitions,
                        # result replicated to every partition of a psum bank.
                        # Bank choice: latest-used slot of the next head's
                        # rotation that this head has already freed, so the
                        # den matmul neither waits on the next head's exp nor
                        # blocks its early QK matmuls.
                        di = len(sbanks) - 1
                        if (nblk - 1) % len(sbanks) == di:
                            di -= 1
                        dps = bank(sbanks[di])
                        nc.tensor.matmul(dps, ones_sb, st['esum'],
                                         start=True, stop=True)
                        dbc = work.tile([128, 512], F32, tag="dbc", bufs=2,
                                        name=f"dbc_{qc}_{hd}")
                        # den in [1, ~4e3]: far from approx_fast edge cases
                        nc.vector.reciprocal_approx_fast(dbc, dps)
                        ct = work.tile([128, 512], MDT, tag="ctx", bufs=9,
                                       name=f"ctx_{qc}_{hd}")
                        nc.vector.tensor_mul(ct, st['ctxps'], dbc)  # frees bank
                        ctx_t.append(ct)

                    hu = [lambda i=i, kb=kb, coff=coff, kb_iter=kb_iter:
                          kb_iter(i, kb, coff)
                          for i, (kb, coff) in enumerate(blocks)]
                    hu.append(tail_flush)
                    heads.append((hu, tail_norm))
                # weave: head h's den/normalize chain is emitted after head
                # h+1's first two QK units, so the in-order PE absorbs the
                # esum-chain wait with useful matmuls.
                units = []
                prev_norm = None
                for hu, tnorm in heads:
                    units += hu[:2]
                    if prev_norm is not None:
                        units.append(prev_norm)
                    units += hu[2:]
                    prev_norm = tnorm
                units.append(prev_norm)
                return units, ctx_t

            # ---- o_proj units: stationary-reuse order, grouped banks ---
            # pipeline>1 runs that many accumulation groups (1 ob each, own
            # bank) in a wavefront, so each group's hd=3 stop-matmul — the
            # one gated on the last head's normalize chain — sits several
            # units later in PE program order.
            def oproj_units(qc, ctx_t, banks, pipeline=1, evict_eng='alt',
                            taper_halves=False, dma_scalar=False,
                            wave_dma=False):
                units = []
                ng = len(banks)
                if pipeline > 1:
                    groups = [(qb, [ob]) for qb in range(4) for ob in range(4)]
                    nob = 1
                else:
                    groups = [(qb, list(range(og, og + ng)))
                              for qb in range(4) for og in range(0, 4, ng)]
                    nob = ng

                def mm(hd, qb, obs, st2, b0):
                    if hd == 0:
                        st2['ops'] = {ob: bank(banks[(b0 + x) % ng])
                                      for x, ob in enumerate(obs)}
                    for ob in obs:
                        nc.tensor.matmul(
                            st2['ops'][ob],
                            ctx_t[hd][:, qb * 128:(qb + 1) * 128],
                            wo_sb[:, hd, ob * 512:(ob + 1) * 512],
                            start=(hd == 0), stop=(hd == GH - 1))

                def evict(qc_, qb, obs, st2, halves=False, force_vec=False):
                    if wave_dma and len(obs) == 4:
                        # pipeline=1 group = one full row-block: stage the 4
                        # banks into one contiguous tile, ONE output DMA
                        # (saves 3 x ~0.6us sync issue slots that would
                        # otherwise contend with the A2 xt-supply DMAs)
                        rows = slice((qc_ * 4 + qb) * 128,
                                     (qc_ * 4 + qb + 1) * 128)
                        wt = work.tile([128, 2048], ODT, tag="wave", bufs=2,
                                       name=f"wave_{qc_}_{qb}")
                        for ob in obs:
                            eng = (nc.scalar.copy if ob % 2 == 0
                                   else nc.vector.tensor_copy)
                            eng(wt[:, ob * 512:(ob + 1) * 512], st2['ops'][ob])
                        nc.sync.dma_start(out=out[rows, :], in_=wt)
                        return
                    for x, ob in enumerate(obs):
                        if evict_eng == 'vector' or force_vec:
                            eng = nc.vector.tensor_copy
                        elif (qb + ob) % 2 == 0:
                            eng = nc.scalar.copy
                        else:
                            eng = nc.vector.tensor_copy
                        rows = slice((qc_ * 4 + qb) * 128,
                                     (qc_ * 4 + qb + 1) * 128)
                        if halves:
                            # separate half-tiles so Act and DVE copy
                            # concurrently; both copies emitted before the
                            # DMAs; DMAs on separate issue queues
                            oths = []
                            for qi, h0 in enumerate((0, 256)):
                                oth = work.tile([128, 256], ODT, tag="outh",
                                                bufs=2,
                                                name=f"oth_{qc_}_{qb}_{ob}_{qi}")
                                hs = slice(h0, h0 + 256)
                                ceng = (nc.scalar.copy if qi == 0
                                        else nc.vector.tensor_copy)
                                ceng(oth, st2['ops'][ob][:, hs])
                                oths.append(oth)
                            for qi, h0 in enumerate((0, 256)):
                                # final halves on two separate issue queues
                                deng = nc.scalar if qi == 0 else nc.sync
                                deng.dma_start(
                                    out=out[rows,
                                            ob * 512 + h0:ob * 512 + h0 + 256],
                                    in_=oths[qi])
                        else:
                            ot = work.tile([128, 512], ODT, tag="outsb",
                                           bufs=4, name=f"ot_{qc_}_{qb}_{ob}")
                            eng(ot, st2['ops'][ob])
                            deng = (nc.scalar if dma_scalar
                                    and eng is nc.scalar.copy else nc.sync)
                            deng.dma_start(
                                out=out[rows, ob * 512:(ob + 1) * 512],
                                in_=ot)

                def evict_wave(qc_, chunk, sts, last):
                    # pipeline=4 wave = one qb row-block x all 4 obs: stage
                    # the 4 psum banks into one contiguous [128,2048] tile
                    # (copies alternate Act/DVE) and DMA it in ONE transfer
                    # (4KB/partition descriptors, 1 issue instead of 4). The
                    # final wave splits across two issue queues so its
                    # completion isn't gated on one 1MB transfer.
                    qb = chunk[0][0]
                    rows = slice((qc_ * 4 + qb) * 128, (qc_ * 4 + qb + 1) * 128)
                    wt = work.tile([128, 2048], ODT, tag="wave", bufs=2,
                                   name=f"wave_{qc_}_{qb}")
                    for (qb_, obs), (st2, b0) in zip(chunk, sts):
                        ob = obs[0]
                        eng = (nc.scalar.copy if ob % 2 == 0
                               else nc.vector.tensor_copy)
                        eng(wt[:, ob * 512:(ob + 1) * 512], st2['ops'][ob])
                    if last:
                        nc.scalar.dma_start(out=out[rows, 0:1024],
                                            in_=wt[:, 0:1024])
                        nc.sync.dma_start(out=out[rows, 1024:2048],
                                          in_=wt[:, 1024:2048])
                    else:
                        nc.sync.dma_start(out=out[rows, :], in_=wt)

                nchunk = (len(groups) + pipeline - 1) // pipeline
                for ci, c0 in enumerate(range(0, len(groups), pipeline)):
                    chunk = groups[c0:c0 + pipeline]
                    sts = [({}, (c0 + gi) * nob) for gi in range(len(chunk))]
                    last_chunk = ci == nchunk - 1
                    for hd in range(GH):
                        for (qb, obs), (st2, b0) in zip(chunk, sts):
                            units.append(
                                lambda hd=hd, qb=qb, obs=obs, st2=st2, b0=b0:
                                mm(hd, qb, obs, st2, b0))
                    if wave_dma:
                        units.append(lambda chunk=chunk, sts=sts,
                                     lc=last_chunk:
                                     evict_wave(qc, chunk, sts, lc))
                        continue
                    for (qb, obs), (st2, b0) in zip(chunk, sts):
                        hv = taper_halves and last_chunk
                        # second-to-last chunk evicts on DVE so Act is idle
                        # when the final halves need it
                        fv = taper_halves and ci == nchunk - 2
                        units.append(lambda qb=qb, obs=obs, st2=st2, hv=hv,
                                     fv=fv:
                                     evict(qc, qb, obs, st2, halves=hv,
                                           force_vec=fv))
                return units

            # ---- emit ---------------------------------------------------
            def splice(main, filler, lead=0):
                # lead: number of main units to run before filler starts
                na, nb = len(filler), len(main)
                ai = 0
                for i, u in enumerate(main):
                    u()
                    eff = max(0, i + 1 - lead)
                    den = max(1, nb - lead)
                    tgt = min(na, eff * na // den) if nb else na
                    while ai < tgt:
                        filler[ai]()
                        ai += 1
                while ai < na:
                    filler[ai]()
                    ai += 1

            ctxs = {}
            if variant != "causal":
                # full attention: every q chunk needs all K chunks, so all
                # projections must complete before the attention stream.
                for sc in range(SC):
                    for u in a2_units(sc, reuse=sc0_get if sc == 0 else None,
                                      wide=True):
                        u()
                for qc in range(SC):
                    bu, ctxs[qc] = b_units(qc, sps_banks=(4, 5, 7),
                                           ctx_banks=(6,))
                    bu += oproj_units(qc, ctxs[qc], banks=(0, 1, 2, 3))
                    for u in bu:
                        u()
            else:
                for u in a2_units(0, reuse=sc0_get, wide=True):
                    u()
                for qc in range(SC - 1):
                    # A2 filler only holds banks 0/1 now: banks 2 and 3 are
                    # free for a 4-deep sps rotation and a second ctx bank.
                    bu, ctxs[qc] = b_units(qc, sps_banks=(4, 5, 7, 2),
                                           ctx_banks=(6, 3))
                    if qc < 2:
                        bu += oproj_units(qc, ctxs[qc], banks=(4, 5, 6, 7))
                    splice(bu, a2_units(qc + 1))
                # qc=3: no A2 filler; defer qc=2's o_proj into this stream,
                # software-pipelined so each group's ct[3]-gated stop-matmul
                # sits late in PE program order. sps must avoid bank 6
                # (qc2's ctx bank: its normalize chain drains late and would
                # stall the in-order PE on the WAR dep); ctx on banks 2/3
                # (A2 pss banks, free since mid-qc2).
                # bank 6 goes to op2's rotation (not the late-head sps
                # rotation): at qc3's ~900ns/block spliced pace a 3-deep sps
                # rotation has slack, while op2's 2-bank wave turnaround was
                # stalling ~300ns per wave (2.4us total).
                bu, ctxs[3] = b_units(3, sps_banks=(4, 5, 7), ctx_banks=(2, 3))
                op2 = oproj_units(2, ctxs[2], banks=(0, 1, 6), pipeline=2,
                                  evict_eng='alt')
                splice(bu, op2[:-6])
                for u in op2[-6:]:
                    u()
                # all non-ctx banks are drained by now; pipeline=4 makes each
                # wave one full row-block (qb x all obs) so it evicts into a
                # contiguous staging tile and DMAs once per wave.
                for u in oproj_units(3, ctxs[3], banks=(0, 1, 6, 4, 5, 7),
                                     pipeline=4, wave_dma=True):
                    u()
    nc.compile()
    return nc


_CACHE = {}


def _get(variant, dt=None):
    dt = dt or DTYPE
    if (variant, dt) not in _CACHE:
        _CACHE[(variant, dt)] = _build(variant, dt)
    return _CACHE[(variant, dt)]


def _rope_tables():
    inv = 1.0 / (10000.0 ** (np.arange(0, D, 2, dtype=np.float64) / D))  # [64]
    t = np.arange(S, dtype=np.float64)
    fr = np.outer(inv, t)                       # [64, S]
    cosT = np.concatenate([np.cos(fr), np.cos(fr)], 0).astype(np.float32)
    # partition-swapped sign-folded sin: rows 0:64 = +sin, rows 64:128 = -sin
    sinT = np.concatenate([np.sin(fr), -np.sin(fr)], 0).astype(np.float32)
    return cosT, sinT


def _btpl_causal():
    # additive triangle mask template: NEG where k > c (128x128)
    k = np.arange(128)[:, None]
    c = np.arange(128)[None, :]
    return np.where(k > c, np.float32(NEG), np.float32(0.0)).astype(np.float32)


def _np_cast(a, dt):
    if dt == "f16":
        return a.astype(np.float16)
    if dt == "bf16":
        import ml_dtypes
        return a.astype(ml_dtypes.bfloat16)
    return a


def _numpy_fallback(hs, Wq, Wk, Wv, Wo, mask):
    B = hs.shape[0]
    cosT, sinT = _rope_tables()
    cos = cosT.T[None, :, None, :]
    sin = np.abs(sinT).T[None, :, None, :]
    outs = []
    for b in range(B):
        x = hs[b]
        q = (x @ Wq).reshape(S, 16, D)[None]
        k = (x @ Wk).reshape(S, 16, D)[None]
        vv = (x @ Wv).reshape(S, 16, D)

        def rope(z):
            z1, z2 = z[..., :64], z[..., 64:]
            rot = np.concatenate([-z2, z1], -1)
            return z * cos + rot * sin

        q, k = rope(q)[0], rope(k)[0]
        o = np.empty((S, 16, D), np.float32)
        m = mask[0, 0]
        for h in range(16):
            sc = (q[:, h] @ k[:, h].T) * SCALE
            sc = np.where(m == 0, -np.inf, sc)
            sc -= sc.max(-1, keepdims=True)
            p = np.exp(sc)
            p /= p.sum(-1, keepdims=True)
            o[:, h] = p @ vv[:, h]
        outs.append(o.reshape(S, HID) @ Wo)
    return np.stack(outs).astype(np.float32)


def kernel(hidden_states, Wq, Wk, Wv, Wo, attention_mask):
    hs = np.asarray(hidden_states, dtype=np.float32)
    Wq, Wk, Wv, Wo = (np.asarray(w, dtype=np.float32) for w in (Wq, Wk, Wv, Wo))
    mask = np.asarray(attention_mask)
    B = hs.shape[0]

    m3 = mask.reshape(-1, mask.shape[-2], mask.shape[-1])
    m2 = m3[0]
    same = all(np.array_equal(m2, m3[i]) for i in range(1, m3.shape[0]))
    if not same:
        return _numpy_fallback(hs, Wq, Wk, Wv, Wo, mask)
    if np.all(m2 == 1):
        variant = "full"
    elif np.array_equal(m2 != 0, np.tril(np.ones((S, S), dtype=bool))):
        variant = "causal"
    else:
        return _numpy_fallback(hs, Wq, Wk, Wv, Wo, mask)

    cosT, sinT = _rope_tables()
    btpl = _btpl_causal() if variant == "causal" else np.zeros((128, 128), np.float32)

    in_maps = []
    for c in range(NCORES):
        b, g = divmod(c, GH)
        gsl = slice(g * GW, (g + 1) * GW)
        in_maps.append({
            "xt": _np_cast(np.ascontiguousarray(hs[b].T), DTYPE),
            "wq": _np_cast(np.ascontiguousarray(Wq[:, gsl]), DTYPE),
            "wk": _np_cast(np.ascontiguousarray(Wk[:, gsl]), DTYPE),
            "wv": _np_cast(np.ascontiguousarray(Wv[:, gsl]), DTYPE),
            "wo": _np_cast(np.ascontiguousarray(Wo[gsl, :]), DTYPE),
            "cost": _np_cast(cosT, DTYPE), "sint": _np_cast(sinT, DTYPE),
            "btpl": _np_cast(btpl, DTYPE),
        })

    nc = _get(variant)
    res = run_bass_kernel_spmd(nc, in_maps, list(range(NCORES))).results
    out = np.zeros((B, S, HID), np.float32)
    for c in range(NCORES):
        b = c // GH
        out[b] += res[c]["out"]
    return out



# revision 40
# speedup vs baseline: 1.0063x; 1.0025x over previous
"""Trainium2 Bass kernel for nn_Attention (dense transformer attention block).

Full-input contract: kernel(**inputs) takes the unsharded inputs and returns
the full output. 8 NeuronCores: tensor-parallel over head groups (4 heads) x
data-parallel over batch (2); core c = b*4 + g. Per core: q/k/v projections
for its head group, RoPE, causal flash-style attention (transposed-P layout,
softmax without max-subtraction), partial o_proj with its rows of Wo; the 4
partials per batch element are summed on the host (the all-reduce of the
row-sharded o_proj).

Optimizations over the first working version (sim 351us -> 287us; HW
366us -> 317us at 2.4GHz):
 - softmax denominator via DVE exp-sum accumulation + ONE ones-matmul
   partition reduce on PE per (head, q-chunk) (the GpSimd
   partition_all_reduce used before costs 3.6us per call on HW and
   serialized the normalize chains: replacing it won -47us). Each head's
   den/recip/ctx-mul chain is woven after the next head's first two QK
   units so the in-order PE absorbs the esum-chain wait.
 - A1 alternates psum bank quads per sc; A2 is d-major (evict chains
   overlap the next d's matmuls); sc0 xt DMAs split per 128KB piece.
 - NOTE: fp8e4m3 PV (DoubleRow) was tried and REJECTED: fp8 noise on
   v/pexp does not average down (signal shrinks with the same 1/sqrt(n)
   as the noise); measured ~3.5% output error vs the 2% budget.
 - NOTE: HW runs sometimes execute with PE at 2.0GHz (P0 power state)
   instead of 2.4GHz -- wall times inflate ~20% run-to-run; check the
   median matmul duration (379ns warm @2.4 vs 454ns @2.0) to compare.
 - causal diagonal blocks trimmed to their valid column range (QK, exp,
   mask add, PV all operate on [128j:512) only); diag blocks ordered last
   so each head's tail exp/esum chain is short.
 - deep PV pend pipeline (8 blocks): exp-gated PV matmuls never stall the
   in-order PE mid-stream; they drain as pure-PE bursts.
 - A2 projection split into two d-pair passes on psum banks {0,1} only
   (same cycles), freeing banks 2/3 for a 4-deep sps rotation and a second
   ctx bank in the attention streams.
 - ctx normalization multiply reads the PSUM bank directly (no scalar
   copy); reciprocal via the fast approx DVE op.
 - o_proj software-pipelined over grouped PSUM banks with stationary
   reuse; qc=2's o_proj deferred into the qc=3 stream as PE filler (qc=3
   has no projection filler and is exp-latency-bound); final group's
   evict+DMA split in halves across Act/DVE copies and both DMA issue
   queues to shorten the end drain.
 - first xt/wv DMAs split into 128KB pieces, xt prefetch pipelined, sc=0
   xt tiles reused by A2(0); fp16 output (host accumulates in fp32).
 - dummy warmup matmuls burn the PE clock-ramp during the DMA-idle start.

The Q/K projection work (phase A2) is interleaved into the attention stream
(phase B) as PE filler: B-qc only needs Q/K columns of chunks <= qc, so
A2-sc(qc+1) runs alongside B-qc, hiding the exp/softmax latency chains.

Matmul dtype configurable (DTYPE): fp16 default (~7e-4 rel err), f32r
fallback (~3.5e-4).
"""
import contextlib
import numpy as np
import concourse.bass as bass
from concourse import bacc, bass_isa
import concourse.mybir as mybir
import concourse.tile as tile
from concourse.bass_utils import run_bass_kernel_spmd

F32 = mybir.dt.float32
F32R = mybir.dt.float32r
F16 = mybir.dt.float16
BF16 = mybir.dt.bfloat16
F8 = mybir.dt.float8e4
DR = mybir.MatmulPerfMode.DoubleRow
EXP = mybir.ActivationFunctionType.Exp
MMDT = {"f32r": F32R, "f16": F16, "bf16": BF16}
# fp8 PV (DoubleRow) was tried and REJECTED: per-element fp8e4m3 noise on
# v/pexp does not average down (signal shrinks with the same 1/sqrt(n) as
# the noise) -- measured ~3.5%% output error vs the 2%% budget. Keep f16.
PV8 = False
DR_ON = False

S = 2048
HID = 2048
D = 128
GH = 4            # heads per core
GW = GH * D       # 512
NCORES = 8
SC = S // 512     # 4 column chunks
HC = HID // 128   # 16 contraction chunks
SCALE = float(D) ** -0.5
NEG = -30000.0   # must be fp16-representable; exp(SCALE*(NEG+score)) == 0

DTYPE = "f16"     # matmul dtype: 'f16' | 'bf16' | 'f32r'


def _build(variant, dt):
    MDT = MMDT[dt]
    two_byte = dt in ("f16", "bf16")
    IDT = MDT if two_byte else F32
    nc = bacc.Bacc("TRN2", target_bir_lowering=False, debug=False,
                   num_devices=NCORES)
    xt = nc.dram_tensor("xt", [HID, S], IDT, kind="ExternalInput").ap()
    wq = nc.dram_tensor("wq", [HID, GW], IDT, kind="ExternalInput").ap()
    wk = nc.dram_tensor("wk", [HID, GW], IDT, kind="ExternalInput").ap()
    wv = nc.dram_tensor("wv", [HID, GW], IDT, kind="ExternalInput").ap()
    wo = nc.dram_tensor("wo", [GW, HID], IDT, kind="ExternalInput").ap()
    cost = nc.dram_tensor("cost", [D, S], IDT, kind="ExternalInput").ap()
    sint = nc.dram_tensor("sint", [D, S], IDT, kind="ExternalInput").ap()
    btpl = nc.dram_tensor("btpl", [D, D], IDT, kind="ExternalInput").ap()
    # fp16 output halves the out-DMA traffic; host accumulates in fp32
    ODT = MDT if two_byte else F32
    out = nc.dram_tensor("out", [S, HID], ODT, kind="ExternalOutput").ap()

    def _bc(ap):
        return ap if two_byte else ap.bitcast(F32R)

    xt_r = _bc(xt.rearrange("(c p) s -> p c s", p=128))   # [128, 16, 2048]
    wq_r = _bc(wq.rearrange("(c p) m -> p c m", p=128))   # [128, 16, 512]
    wk_r = _bc(wk.rearrange("(c p) m -> p c m", p=128))
    wv_r = _bc(wv.rearrange("(c p) m -> p c m", p=128))
    wo_r = _bc(wo.rearrange("(c p) m -> p c m", p=128))   # [128, 4, 2048]

    XB = 4                   # h-chunks per xt DMA
    NXT = HC // XB           # 4 xt tiles per s-chunk

    with tile.TileContext(nc) as tc:
        with contextlib.ExitStack() as ctx:
            persist = ctx.enter_context(tc.tile_pool(name="persist", bufs=1))
            psum = ctx.enter_context(tc.tile_pool(name="psum", bufs=1, space="PSUM"))
            work = ctx.enter_context(tc.tile_pool(name="work", bufs=1))

            _n = [0]

            def bank(i, shape=(128, 512)):
                _n[0] += 1
                return psum.tile(list(shape), F32, tag=f"b{i}", name=f"bk{i}_{_n[0]}")

            qts = [[persist.tile([128, 512], MDT, tag=f"qt{h}_{s}",
                                 name=f"qt{h}_{s}") for s in range(SC)]
                   for h in range(GH)]
            kts = [[persist.tile([128, 512], MDT, tag=f"kt{h}_{s}",
                                 name=f"kt{h}_{s}") for s in range(SC)]
                   for h in range(GH)]
            VDT = F8 if PV8 else MDT
            # v stored as k-block PAIRS [128, 2, GW]: slice [:, :, hd*128..]
            # is the [128, 2, 128] DoubleRow stationary operand
            vts2 = [persist.tile([128, 2, GW], VDT, tag=f"v{j}", name=f"v{j}")
                    for j in range(HC // 2)]
            cos_sb = persist.tile([128, S], MDT, tag="cos")
            sin_sb = persist.tile([128, S], MDT, tag="sin")
            btpl_sb = persist.tile([128, D], MDT, tag="btpl")
            wo_sb = persist.tile([128, GH, HID], MDT, tag="wo")
            # all-ones stationary operand: ones^T @ esum = partition-sum of
            # esum replicated across all 128 partitions (the softmax
            # denominator reduce, on PE instead of the 3.6us GpSimd op)
            ones_sb = persist.tile([128, D], MDT, tag="ones")

            def xt_tile(sc, j):
                t = work.tile([128, XB, 512], MDT, tag="xt", bufs=9,
                              name=f"xt_{sc}_{j}")
                if sc == 0:
                    # per-chunk pieces during the DMA ramp: subtile deps let
                    # the first consumer start when its 128KB piece lands
                    for hh in range(XB):
                        nc.sync.dma_start(
                            out=t[:, hh, :],
                            in_=xt_r[:, j * XB + hh, 0:512])
                else:
                    nc.sync.dma_start(
                        out=t, in_=xt_r[:, j * XB:(j + 1) * XB,
                                        sc * 512:(sc + 1) * 512])
                return t

            # weights for Q/K/O + tables: DMAs deferred into the A1 stream
            # (issued from the scalar engine after each sc's vts copies) so
            # the congested startup window belongs to the xt/wv stream alone.
            wq_sb = work.tile([128, HC, GW], MDT, tag="wqk", bufs=2)
            wk_sb = work.tile([128, HC, GW], MDT, tag="wqk", bufs=2)

            # ---- A1: V = X @ Wv (banks b0..b3) -------------------------
            # First j-chunk of xt/wv split per h-chunk so the first matmul
            # only waits on 2x128KB of DMA. Remaining xt tiles issue
            # interleaved with the wv chunks, kept 3 iterations ahead.
            with tc.tile_pool(name="phV", bufs=1) as phv:
                xt0 = []
                wv0 = []
                for hh in range(XB):
                    xt0_t = work.tile([128, 512], MDT, tag="xt0", bufs=XB,
                                      name=f"xt0_{hh}")
                    nc.sync.dma_start(out=xt0_t, in_=xt_r[:, hh, 0:512])
                    xt0.append(xt0_t)
                    wv0_t = phv.tile([128, GW], MDT, tag="wvf0", bufs=XB,
                                     name=f"wvf0_{hh}")
                    nc.sync.dma_start(out=wv0_t, in_=wv_r[:, hh, :])
                    wv0.append(wv0_t)
                order = [(sc, j) for sc in range(SC) for j in range(NXT)][1:]
                pend = {}
                wv_cs = [None] * NXT

                def issue(idx):
                    if idx < len(order):
                        pend[order[idx]] = xt_tile(*order[idx])
                    if idx < NXT - 1:
                        wvc = phv.tile([128, XB, GW], MDT, tag="wvf",
                                       bufs=NXT - 1, name=f"wvf_{idx + 1}")
                        nc.sync.dma_start(
                            out=wvc,
                            in_=wv_r[:, (idx + 1) * XB:(idx + 2) * XB, :])
                        wv_cs[idx + 1] = wvc

                for idx in range(3):
                    issue(idx)
                nxt_issue = 3
                # dummy matmuls on a memset tile: burn the PE clock-ramp
                # (half speed for the first ~3us of activity) and the
                # DMA-supply bubbles at the start, where PE is idle anyway.
                warm = work.tile([128, 512], MDT, tag="warm", bufs=1)
                nc.vector.memset(warm, 0.0)
                nc.vector.memset(ones_sb, 1.0)

                def warmup(n):
                    wb = bank(5)
                    for _ in range(n):
                        nc.tensor.matmul(wb, warm[:, 0:128], warm,
                                         start=True, stop=True)

                warmup(8)
                sc0_xt = {}
                for sc in range(SC):
                    # alternate bank quads per sc: the next sc's first matmul
                    # never waits on this sc's eviction copies
                    vps = [bank(st + 4 * (sc % 2)) for st in range(4)]
                    for j in range(NXT):
                        if sc == 0 and j == 1:
                            # 10 warm matmuls (~2.1us warm) fill the xt j=1
                            # DMA-ramp stall so HAM never re-throttles; the
                            # observed gap is always >2.4us, so these never
                            # delay real work
                            warmup(10)
                        if sc == 0 and j == 0:
                            xts = xt0
                            wvs = wv0
                        else:
                            xt_t = pend.pop((sc, j))
                            if sc == 0:
                                sc0_xt[j] = xt_t
                            issue(nxt_issue)
                            nxt_issue += 1
                            xts = [xt_t[:, hh, :] for hh in range(XB)]
                            wvs = ([wv0[hh] for hh in range(XB)] if j == 0
                                   else [wv_cs[j][:, hh, :] for hh in range(XB)])
                        for hh in range(XB):
                            h = j * XB + hh
                            for st in range(4):
                                nc.tensor.matmul(
                                    vps[st], xts[hh][:, st * 128:(st + 1) * 128],
                                    wvs[hh],
                                    start=(h == 0), stop=(h == HC - 1))
                                if h == HC - 1:
                                    # evict bank st while PE finishes st+1..3,
                                    # so the next sc's first matmul on bank 0
                                    # doesn't wait for the whole copy batch
                                    kb = sc * 4 + st
                                    nc.scalar.copy(
                                        vts2[kb // 2][:, kb % 2, :], vps[st])

            def sc0_get(j, hh):
                return xt0[hh] if j == 0 else sc0_xt[j][:, hh, :]

            nc.sync.dma_start(out=wq_sb, in_=wq_r)
            nc.sync.dma_start(out=wk_sb, in_=wk_r)
            nc.sync.dma_start(out=wo_sb, in_=wo_r)
            nc.sync.dma_start(out=cos_sb, in_=_bc(cost))
            nc.sync.dma_start(out=sin_sb, in_=_bc(sint))
            nc.sync.dma_start(out=btpl_sb, in_=_bc(btpl))

            # ---- A2 units: one s-chunk = Q half then K half ------------
            # reuse: (j, hh) -> AP accessor for already-resident xt tiles
            # (sc=0 reuses A1's tiles — saves 2MB of DMA in the congested
            # startup window).
            # wide=True uses 4 psum banks (one pass per half); wide=False
            # does two d-pair passes on banks {0,1} only, freeing banks 2/3
            # for the attention streams (deeper sps rotation + 2 ctx banks).
            # The pass-boundary evict latency is absorbed by the B-stream
            # units spliced between A2 units.
            def a2_units(sc, reuse=None, wide=False):
                ssl = slice(sc * 512, (sc + 1) * 512)
                units = []
                state = {}

                def prep():
                    if reuse is not None:
                        state['get'] = reuse
                    else:
                        tiles = [xt_tile(sc, j) for j in range(NXT)]
                        state['get'] = lambda j, hh: tiles[j][:, hh, :]

                units.append(prep)

                def evict(d, dsl, pss):
                    # RoPE reads the PSUM bank directly (DVE can read PSUM):
                    # no Act copy hop — frees ~22us of Act time globally so
                    # the exp stream never queues behind t2 copies; d-major
                    # ordering gives the bank 3.4us of slack vs this ~1.5us
                    # DVE chain.
                    src = pss[d]
                    t1 = work.tile([128, 512], MDT, tag="t1", bufs=3,
                                   name=f"t1_{sc}_{d}_{dsl.tensor.name}")
                    nc.vector.tensor_mul(t1, src, cos_sb[:, ssl])
                    nc.vector.tensor_mul(dsl[0:64, :], src[64:128, :],
                                         sin_sb[64:128, ssl])
                    nc.vector.tensor_mul(dsl[64:128, :], src[0:64, :],
                                         sin_sb[0:64, ssl])
                    nc.vector.tensor_add(dsl, dsl, t1)

                dgroups = ([tuple(range(GH))] if wide
                           else [(0, 1), (2, 3)])
                for half, (w_sb, dts) in enumerate(
                        ((wq_sb, [qts[d][sc] for d in range(GH)]),
                         (wk_sb, [kts[d][sc] for d in range(GH)]))):
                    for dg in dgroups:
                        pss = {}
                        # d-major: each d's 16 accumulation steps complete
                        # before the next d starts, so its evict chain (Act
                        # copy + DVE RoPE) overlaps the next d's matmuls and
                        # the pass boundary never stalls on an eviction.
                        for x, d in enumerate(dg):

                            def stepd(j, hh, d=d, x=x, w_sb=w_sb, pss=pss):
                                h = j * XB + hh
                                if h == 0:
                                    pss[d] = bank(x)
                                nc.tensor.matmul(
                                    pss[d], w_sb[:, h, d * 128:(d + 1) * 128],
                                    state['get'](j, hh),
                                    start=(h == 0), stop=(h == HC - 1))

                            for j in range(NXT):
                                for hh in range(XB):
                                    units.append(lambda j=j, hh=hh,
                                                 stepd=stepd: stepd(j, hh))
                            units.append(lambda d=d, dsl=dts[d], pss=pss:
                                         evict(d, dsl, pss))
                return units

            # ---- B units: attention for one q chunk --------------------
            # Per block: QK matmul (sps bank), optional mask add, exp, DVE
            # exp-sum accumulate, PV accumulate (ctx bank). Tail: GpSimd
            # partition_all_reduce of the exp-sum -> reciprocal -> ctx mul.
            # Returns (units, ctx_t); o_proj emitted separately.
            def b_units(qc, sps_banks, ctx_banks, sps_banks_late=None):
                if variant == "causal":
                    # (kbi, coff): off-diag first, diag last — the head's
                    # final exp/esum-add before the normalize chain is then
                    # only 128 wide, shortening the tail latency. qc=0 has
                    # no off-diag; its j=0 diag is full width and inits psum.
                    blocks = [(kb, 0) for kb in range(4 * qc)]
                    blocks += [(4 * qc + j, 128 * j) for j in range(4)]
                    noff = 4 * qc
                else:
                    blocks = [(kb, 0) for kb in range(HC)]
                    noff = HC
                nblk = len(blocks)
                ctx_t = []
                heads = []
                for hd in range(GH):
                    st = {}

                    def start_head(st=st, hd=hd):
                        st['ctxps'] = bank(ctx_banks[hd % len(ctx_banks)])
                        st['esum'] = work.tile([128, 512], MDT, tag="esum",
                                               bufs=3, name=f"esum_{qc}_{hd}")
                        st['pend'] = []

                    def flush(last, st=st, hd=hd):
                        ent = st['pend'].pop(0)
                        if ent[0] == 'dr':
                            # off-diag pair: one DoubleRow matmul covers both
                            # 128-row k-blocks of the pair tile
                            _, pex, kbp2, first = ent
                            nc.tensor.matmul(
                                st['ctxps'][:, 0:512],
                                vts2[kbp2][:, :, hd * 128:(hd + 1) * 128],
                                pex[:, :, 0:512],
                                start=first, stop=last, perf_mode=DR)
                        else:
                            _, pex, par, kbp, coff, first = ent
                            nc.tensor.matmul(
                                st['ctxps'][:, coff:512],
                                vts2[kbp // 2][:, kbp % 2,
                                               hd * 128:(hd + 1) * 128],
                                pex[:, par, coff:512],
                                start=first, stop=last)

                    sbanks = (sps_banks_late
                              if hd > 0 and sps_banks_late else sps_banks)

                    def kb_iter(i, kb, coff, st=st, hd=hd, sbanks=sbanks,
                                start_head=start_head, flush=flush):
                        if i == 0:
                            start_head()
                        sps = bank(sbanks[i % len(sbanks)])
                        diag = variant == "causal" and kb >= 4 * qc
                        nc.tensor.matmul(
                            sps[:, coff:512],
                            kts[hd][kb // 4][:, (kb % 4) * 128:(kb % 4 + 1) * 128],
                            qts[hd][qc][:, coff:512],
                            start=True, stop=True)
                        if diag:
                            nc.vector.tensor_add(sps[:, coff:coff + 128],
                                                 sps[:, coff:coff + 128],
                                                 btpl_sb)
                        par = i % 2
                        if par == 0:
                            st['pex'] = work.tile([128, 2, 512], VDT,
                                                  tag="pexp", bufs=6,
                                                  name=f"pexp_{qc}_{hd}_{kb}")
                        pex = st['pex']
                        nc.scalar.activation(pex[:, par, coff:512],
                                             sps[:, coff:512], EXP, scale=SCALE)
                        if i == 0:
                            nc.vector.tensor_copy(st['esum'], pex[:, 0, :])
                        else:
                            nc.vector.tensor_add(st['esum'][:, coff:512],
                                                 st['esum'][:, coff:512],
                                                 pex[:, par, coff:512])
                        ent = None
                        if PV8 and DR_ON and i < noff:
                            if par == 1:
                                ent = ('dr', pex, kb // 2, i == 1)
                        else:
                            ent = ('sg', pex, par, kb, coff, i == 0)
                        if ent is not None:
                            if len(st['pend']) >= 4:
                                flush(False)
                            st['pend'].append(ent)

                    def tail_flush(st=st, flush=flush):
                        while len(st['pend']) > 1:
                            flush(False)
                        flush(True)

                    def tail_norm(st=st, hd=hd, sbanks=sbanks):
                        # denominator: ones^T @ esum sums the 128 partitions,
                        # result replicated to every partition of a psum bank.
                        # Bank choice: latest-used slot of the next head's
                        # rotation that this head has already freed, so the
                        # den matmul neither waits on the next head's exp nor
                        # blocks its early QK matmuls.
                        di = len(sbanks) - 1
                        if (nblk - 1) % len(sbanks) == di:
                            di -= 1
                        dps = bank(sbanks[di])
                        nc.tensor.matmul(dps, ones_sb, st['esum'],
                                         start=True, stop=True)
                        dbc = work.tile([128, 512], F32, tag="dbc", bufs=2,
                                        name=f"dbc_{qc}_{hd}")
                        # den in [1, ~4e3]: far from approx_fast edge cases
                        nc.vector.reciprocal_approx_fast(dbc, dps)
                        ct = work.tile([128, 512], MDT, tag="ctx", bufs=9,
                                       name=f"ctx_{qc}_{hd}")
                        nc.vector.tensor_mul(ct, st['ctxps'], dbc)  # frees bank
                        ctx_t.append(ct)

                    hu = [lambda i=i, kb=kb, coff=coff, kb_iter=kb_iter:
                          kb_iter(i, kb, coff)
                          for i, (kb, coff) in enumerate(blocks)]
                    hu.append(tail_flush)
                    heads.append((hu, tail_norm))
                # weave: head h's den/normalize chain is emitted after head
                # h+1's first two QK units, so the in-order PE absorbs the
                # esum-chain wait with useful matmuls.
                units = []
                prev_norm = None
                for hu, tnorm in heads:
                    units += hu[:2]
                    if prev_norm is not None:
                        units.append(prev_norm)
                    units += hu[2:]
                    prev_norm = tnorm
                units.append(prev_norm)
                return units, ctx_t

            # ---- o_proj units: stationary-reuse order, grouped banks ---
            # pipeline>1 runs that many accumulation groups (1 ob each, own
            # bank) in a wavefront, so each group's hd=3 stop-matmul — the
            # one gated on the last head's normalize chain — sits several
            # units later in PE program order.
            def oproj_units(qc, ctx_t, banks, pipeline=1, evict_eng='alt',
                            taper_halves=False, dma_scalar=False,
                            wave_dma=False):
                units = []
                ng = len(banks)
                if pipeline > 1:
                    groups = [(qb, [ob]) for qb in range(4) for ob in range(4)]
                    nob = 1
                else:
                    groups = [(qb, list(range(og, og + ng)))
                              for qb in range(4) for og in range(0, 4, ng)]
                    nob = ng

                def mm(hd, qb, obs, st2, b0):
                    if hd == 0:
                        st2['ops'] = {ob: bank(banks[(b0 + x) % ng])
                                      for x, ob in enumerate(obs)}
                    for ob in obs:
                        nc.tensor.matmul(
                            st2['ops'][ob],
                            ctx_t[hd][:, qb * 128:(qb + 1) * 128],
                            wo_sb[:, hd, ob * 512:(ob + 1) * 512],
                            start=(hd == 0), stop=(hd == GH - 1))

                def evict(qc_, qb, obs, st2, halves=False, force_vec=False):
                    if wave_dma and len(obs) == 4:
                        # pipeline=1 group = one full row-block: stage the 4
                        # banks into one contiguous tile, ONE output DMA
                        # (saves 3 x ~0.6us sync issue slots that would
                        # otherwise contend with the A2 xt-supply DMAs)
                        rows = slice((qc_ * 4 + qb) * 128,
                                     (qc_ * 4 + qb + 1) * 128)
                        wt = work.tile([128, 2048], ODT, tag="wave", bufs=2,
                                       name=f"wave_{qc_}_{qb}")
                        for ob in obs:
                            eng = (nc.scalar.copy if ob % 2 == 0
                                   else nc.vector.tensor_copy)
                            eng(wt[:, ob * 512:(ob + 1) * 512], st2['ops'][ob])
                        nc.sync.dma_start(out=out[rows, :], in_=wt)
                        return
                    for x, ob in enumerate(obs):
                        if evict_eng == 'vector' or force_vec:
                            eng = nc.vector.tensor_copy
                        elif (qb + ob) % 2 == 0:
                            eng = nc.scalar.copy
                        else:
                            eng = nc.vector.tensor_copy
                        rows = slice((qc_ * 4 + qb) * 128,
                                     (qc_ * 4 + qb + 1) * 128)
                        if halves:
                            # separate half-tiles so Act and DVE copy
                            # concurrently; both copies emitted before the
                            # DMAs; DMAs on separate issue queues
                            oths = []
                            for qi, h0 in enumerate((0, 256)):
                                oth = work.tile([128, 256], ODT, tag="outh",
                                                bufs=2,
                                                name=f"oth_{qc_}_{qb}_{ob}_{qi}")
                                hs = slice(h0, h0 + 256)
                                ceng = (nc.scalar.copy if qi == 0
                                        else nc.vector.tensor_copy)
                                ceng(oth, st2['ops'][ob][:, hs])
                                oths.append(oth)
                            for qi, h0 in enumerate((0, 256)):
                                # final halves on two separate issue queues
                                deng = nc.scalar if qi == 0 else nc.sync
                                deng.dma_start(
                                    out=out[rows,
                                            ob * 512 + h0:ob * 512 + h0 + 256],
                                    in_=oths[qi])
                        else:
                            ot = work.tile([128, 512], ODT, tag="outsb",
                                           bufs=4, name=f"ot_{qc_}_{qb}_{ob}")
                            eng(ot, st2['ops'][ob])
                            deng = (nc.scalar if dma_scalar
                                    and eng is nc.scalar.copy else nc.sync)
                            deng.dma_start(
                                out=out[rows, ob * 512:(ob + 1) * 512],
                                in_=ot)

                def evict_wave(qc_, chunk, sts, last):
                    # pipeline=4 wave = one qb row-block x all 4 obs: stage
                    # the 4 psum banks into one contiguous [128,2048] tile
                    # (copies alternate Act/DVE) and DMA it in ONE transfer
                    # (4KB/partition descriptors, 1 issue instead of 4). The
                    # final wave splits across two issue queues so its
                    # completion isn't gated on one 1MB transfer.
                    qb = chunk[0][0]
                    rows = slice((qc_ * 4 + qb) * 128, (qc_ * 4 + qb + 1) * 128)
                    wt = work.tile([128, 2048], ODT, tag="wave", bufs=2,
                                   name=f"wave_{qc_}_{qb}")
                    for (qb_, obs), (st2, b0) in zip(chunk, sts):
                        ob = obs[0]
                        eng = (nc.scalar.copy if ob % 2 == 0
                               else nc.vector.tensor_copy)
                        eng(wt[:, ob * 512:(ob + 1) * 512], st2['ops'][ob])
                    if last:
                        nc.scalar.dma_start(out=out[rows, 0:1024],
                                            in_=wt[:, 0:1024])
                        nc.sync.dma_start(out=out[rows, 1024:2048],
                                          in_=wt[:, 1024:2048])
                    else:
                        nc.sync.dma_start(out=out[rows, :], in_=wt)

                nchunk = (len(groups) + pipeline - 1) // pipeline
                for ci, c0 in enumerate(range(0, len(groups), pipeline)):
                    chunk = groups[c0:c0 + pipeline]
                    sts = [({}, (c0 + gi) * nob) for gi in range(len(chunk))]
                    last_chunk = ci == nchunk - 1
                    for hd in range(GH):
                        for (qb, obs), (st2, b0) in zip(chunk, sts):
                            units.append(
                                lambda hd=hd, qb=qb, obs=obs, st2=st2, b0=b0:
                                mm(hd, qb, obs, st2, b0))
                    if wave_dma:
                        units.append(lambda chunk=chunk, sts=sts,
                                     lc=last_chunk:
                                     evict_wave(qc, chunk, sts, lc))
                        continue
                    for (qb, obs), (st2, b0) in zip(chunk, sts):
                        hv = taper_halves and last_chunk
                        # second-to-last chunk evicts on DVE so Act is idle
                        # when the final halves need it
                        fv = taper_halves and ci == nchunk - 2
                        units.append(lambda qb=qb, obs=obs, st2=st2, hv=hv,
                                     fv=fv:
                                     evict(qc, qb, obs, st2, halves=hv,
                                           force_vec=fv))
                return units

            # ---- emit ---------------------------------------------------
            def splice(main, filler, lead=0):
                # lead: number of main units to run before filler starts
                na, nb = len(filler), len(main)
                ai = 0
                for i, u in enumerate(main):
                    u()
                    eff = max(0, i + 1 - lead)
                    den = max(1, nb - lead)
                    tgt = min(na, eff * na // den) if nb else na
                    while ai < tgt:
                        filler[ai]()
                        ai += 1
                while ai < na:
                    filler[ai]()
                    ai += 1

            ctxs = {}
            if variant != "causal":
                # full attention: every q chunk needs all K chunks, so all
                # projections must complete before the attention stream.
                for sc in range(SC):
                    for u in a2_units(sc, reuse=sc0_get if sc == 0 else None,
                                      wide=True):
                        u()
                for qc in range(SC):
                    bu, ctxs[qc] = b_units(qc, sps_banks=(4, 5, 7),
                                           ctx_banks=(6,))
                    bu += oproj_units(qc, ctxs[qc], banks=(0, 1, 2, 3))
                    for u in bu:
                        u()
            else:
                for u in a2_units(0, reuse=sc0_get, wide=True):
                    u()
                for qc in range(SC - 1):
                    # A2 filler only holds banks 0/1 now: banks 2 and 3 are
                    # free for a 4-deep sps rotation and a second ctx bank.
                    bu, ctxs[qc] = b_units(qc, sps_banks=(4, 5, 7, 2),
                                           ctx_banks=(6, 3))
                    if qc < 2:
                        bu += oproj_units(qc, ctxs[qc], banks=(4, 5, 6, 7))
                    splice(bu, a2_units(qc + 1))
                # qc=3: no A2 filler; defer qc=2's o_proj into this stream,
                # software-pipelined so each group's ct[3]-gated stop-matmul
                # sits late in PE program order. sps must avoid bank 6
                # (qc2's ctx bank: its normalize chain drains late and would
                # stall the in-order PE on the WAR dep); ctx on banks 2/3
                # (A2 pss banks, free since mid-qc2).
                # bank 6 goes to op2's rotation (not the late-head sps
                # rotation): at qc3's ~900ns/block spliced pace a 3-deep sps
                # rotation has slack, while op2's 2-bank wave turnaround was
                # stalling ~300ns per wave (2.4us total).
                bu, ctxs[3] = b_units(3, sps_banks=(4, 5, 7), ctx_banks=(2, 3))
                op2 = oproj_units(2, ctxs[2], banks=(0, 1, 6), pipeline=2,
                                  evict_eng='alt')
                splice(bu, op2[:-6])
                for u in op2[-6:]:
                    u()
                # all non-ctx banks are drained by now; pipeline=4 makes each
                # wave one full row-block (qb x all obs) so it evicts into a
                # contiguous staging tile and DMAs once per wave.
                for u in oproj_units(3, ctxs[3], banks=(0, 1, 6, 4, 5, 7),
                                     pipeline=4, wave_dma=True):
                    u()
    nc.compile()
    return nc


_CACHE = {}


def _get(variant, dt=None):
    dt = dt or DTYPE
    if (variant, dt) not in _CACHE:
        _CACHE[(variant, dt)] = _build(variant, dt)
    return _CACHE[(variant, dt)]


def _rope_tables():
    inv = 1.0 / (10000.0 ** (np.arange(0, D, 2, dtype=np.float64) / D))  # [64]
    t = np.arange(S, dtype=np.float64)
    fr = np.outer(inv, t)                       # [64, S]
    cosT = np.concatenate([np.cos(fr), np.cos(fr)], 0).astype(np.float32)
    # partition-swapped sign-folded sin: rows 0:64 = +sin, rows 64:128 = -sin
    sinT = np.concatenate([np.sin(fr), -np.sin(fr)], 0).astype(np.float32)
    return cosT, sinT


def _btpl_causal():
    # additive triangle mask template: NEG where k > c (128x128)
    k = np.arange(128)[:, None]
    c = np.arange(128)[None, :]
    return np.where(k > c, np.float32(NEG), np.float32(0.0)).astype(np.float32)


def _np_cast(a, dt):
    if dt == "f16":
        return a.astype(np.float16)
    if dt == "bf16":
        import ml_dtypes
        return a.astype(ml_dtypes.bfloat16)
    return a


def _numpy_fallback(hs, Wq, Wk, Wv, Wo, mask):
    B = hs.shape[0]
    cosT, sinT = _rope_tables()
    cos = cosT.T[None, :, None, :]
    sin = np.abs(sinT).T[None, :, None, :]
    outs = []
    for b in range(B):
        x = hs[b]
        q = (x @ Wq).reshape(S, 16, D)[None]
        k = (x @ Wk).reshape(S, 16, D)[None]
        vv = (x @ Wv).reshape(S, 16, D)

        def rope(z):
            z1, z2 = z[..., :64], z[..., 64:]
            rot = np.concatenate([-z2, z1], -1)
            return z * cos + rot * sin

        q, k = rope(q)[0], rope(k)[0]
        o = np.empty((S, 16, D), np.float32)
        m = mask[0, 0]
        for h in range(16):
            sc = (q[:, h] @ k[:, h].T) * SCALE
            sc = np.where(m == 0, -np.inf, sc)
            sc -= sc.max(-1, keepdims=True)
            p = np.exp(sc)
            p /= p.sum(-1, keepdims=True)
            o[:, h] = p @ vv[:, h]
        outs.append(o.reshape(S, HID) @ Wo)
    return np.stack(outs).astype(np.float32)


def kernel(hidden_states, Wq, Wk, Wv, Wo, attention_mask):
    hs = np.asarray(hidden_states, dtype=np.float32)
    Wq, Wk, Wv, Wo = (np.asarray(w, dtype=np.float32) for w in (Wq, Wk, Wv, Wo))
    mask = np.asarray(attention_mask)
    B = hs.shape[0]

    m3 = mask.reshape(-1, mask.shape[-2], mask.shape[-1])
    m2 = m3[0]
    same = all(np.array_equal(m2, m3[i]) for i in range(1, m3.shape[0]))
    if not same:
        return _numpy_fallback(hs, Wq, Wk, Wv, Wo, mask)
    if np.all(m2 == 1):
        variant = "full"
    elif np.array_equal(m2 != 0, np.tril(np.ones((S, S), dtype=bool))):
        variant = "causal"
    else:
        return _numpy_fallback(hs, Wq, Wk, Wv, Wo, mask)

    cosT, sinT = _rope_tables()
    btpl = _btpl_causal() if variant == "causal" else np.zeros((128, 128), np.float32)

    in_maps = []
    for c in range(NCORES):
        b, g = divmod(c, GH)
        gsl = slice(g * GW, (g + 1) * GW)
        in_maps.append({
            "xt": _np_cast(np.ascontiguousarray(hs[b].T), DTYPE),
            "wq": _np_cast(np.ascontiguousarray(Wq[:, gsl]), DTYPE),
            "wk": _np_cast(np.ascontiguousarray(Wk[:, gsl]), DTYPE),
            "wv": _np_cast(np.ascontiguousarray(Wv[:, gsl]), DTYPE),
            "wo": _np_cast(np.ascontiguousarray(Wo[gsl, :]), DTYPE),
            "cost": _np_cast(cosT, DTYPE), "sint": _np_cast(sinT, DTYPE),
            "btpl": _np_cast(btpl, DTYPE),
        })

    nc = _get(variant)
    res = run_bass_kernel_spmd(nc, in_maps, list(range(NCORES))).results
    out = np.zeros((B, S, HID), np.float32)
    for c in range(NCORES):
        b = c // GH
        out[b] += res[c]["out"]
    return out

